# revision 9
# baseline (speedup 1.0000x reference)
"""Llama-style transformer block on 8 TRN2 NeuronCores.

Strategy (Megatron tensor-parallel, feature-major activations):
  - Residual stream kept TRANSPOSED (x^T: [D, S], feature-major) so every
    matmul contracts over the partition dim with zero on-chip transposes.
  - Per core: 4 attention heads (512 of 4096 q/k/v dims) and 1376 (padded
    to 1408) of the 11008 FFN hidden dims.
  - RMSNorm: per-core partial sum-of-squares over its 512-feature shard,
    AllReduce [1,2048], scale applied to own shard, AllGather the
    normalized activations (feature-stacked = exactly the layout the
    matmuls consume).
  - Attention: scores computed TRANSPOSED ([s_k, s_q]) so the softmax'd
    probabilities feed the AV matmul directly (contraction over s_k on
    partitions).  Softmax skips the max-subtraction (exact for softmax;
    scores are bounded so exp cannot overflow in fp32/bf16).  Denominators
    come from a ones-vector matmul; normalization is deferred to the
    attention output (columns scaled by 1/sum).
  - RoPE on Q^T/K^T: head dims pre-permuted (evens then odds) on the host
    inside each head's weight columns, so the rotation is two aligned
    half-tile swaps + elementwise mults against [cos;cos] and [-sin;sin]
    tables.
  - wo / w2 outputs are partial sums -> ReduceScatter back to the
    feature-sharded residual.
  - Host gathers the 8 output shards and transposes back.

Compute dtype bf16, fp32 accumulation in PSUM everywhere.
"""

import math

import ml_dtypes
import numpy as np

import concourse.bass as bass
import concourse.mybir as mybir
import concourse.tile as tile
from concourse import bacc
from concourse.bass_utils import run_bass_kernel_spmd

# problem dims
S = 2048
D = 4096
HD = 128
NH = 32
F = 11008
CORES = 8
NHC = NH // CORES          # heads per core = 4
DQ = NHC * HD              # q/k/v dims per core = 512
FC = F // CORES            # ffn dims per core = 1376
FT = 11                    # padded f-tiles per core
FP = FT * 128              # padded ffn dims per core = 1408
EPS = 1e-5
P = 128
NCH = S // 512             # 512-token chunks = 4
DT = D // P                # d tiles = 32
ST = S // P                # s tiles = 16

CDT = mybir.dt.bfloat16
NP_CDT = ml_dtypes.bfloat16

_COMPILED = None
DEBUG = False


def _build():
    nc = bacc.Bacc("TRN2", target_bir_lowering=False, debug=False,
                   num_devices=CORES)
    f32 = mybir.dt.float32

    # ---- kernel I/O ----
    xT_s = nc.declare_dram_parameter("xT_s", [DQ, S], f32, isOutput=False)
    w_qk = nc.declare_dram_parameter("w_qk", [8, P, DT, P], CDT, isOutput=False)
    w_v = nc.declare_dram_parameter("w_v", [DT, P, DQ], CDT, isOutput=False)
    w_o = nc.declare_dram_parameter("w_o", [P, 32, 4, P], CDT, isOutput=False)
    w_1 = nc.declare_dram_parameter("w_1", [FT, P, DT, P], CDT, isOutput=False)
    w_3 = nc.declare_dram_parameter("w_3", [FT, P, DT, P], CDT, isOutput=False)
    w_2 = nc.declare_dram_parameter("w_2", [32, P, FT, P], CDT, isOutput=False)
    cos2 = nc.declare_dram_parameter("cos2", [P, S], CDT, isOutput=False)
    sinsg2 = nc.declare_dram_parameter("sinsg2", [P, S], CDT, isOutput=False)
    dmask = nc.declare_dram_parameter("dmask", [P, P], f32, isOutput=False)
    outT_s = nc.declare_dram_parameter("outT_s", [DQ, S], f32, isOutput=True)
    dbg = {}
    if DEBUG:
        for nm, shp, dt_ in [("dbg_zT", [D, S], CDT), ("dbg_qt", [DQ, S], CDT),
                             ("dbg_kt", [DQ, S], CDT), ("dbg_v", [P, ST, DQ], CDT),
                             ("dbg_attnT", [P, NHC, S], CDT),
                             ("dbg_yT", [D, S], CDT), ("dbg_yrs", [DQ, S], CDT),
                             ("dbg_h", [DQ, S], f32), ("dbg_hnT", [D, S], CDT),
                             ("dbg_oT", [D, S], CDT), ("dbg_ors", [DQ, S], CDT)]:
            dbg[nm] = nc.declare_dram_parameter(nm, shp, dt_, isOutput=True)

    # ---- internal DRAM (collective bounce buffers & spill) ----
    ssq1_in = nc.dram_tensor("ssq1_in", [1, S], f32)
    ssq1_out = nc.dram_tensor("ssq1_out", [1, S], f32, addr_space="Shared")
    zs_cc = nc.dram_tensor("zs_cc", [DQ, S], CDT)
    zT_ag = nc.dram_tensor("zT_ag", [D, S], CDT, addr_space="Shared")
    qt_dram = nc.dram_tensor("qt_dram", [DQ, S], CDT)
    kt_dram = nc.dram_tensor("kt_dram", [DQ, S], CDT)
    sums_dram = nc.dram_tensor("sums_dram", [16, 512], f32)
    yT_cc = nc.dram_tensor("yT_cc", [D, S], CDT)
    y_rs = nc.dram_tensor("y_rs", [DQ, S], CDT)
    ssq2_in = nc.dram_tensor("ssq2_in", [1, S], f32)
    ssq2_out = nc.dram_tensor("ssq2_out", [1, S], f32, addr_space="Shared")
    hn_cc = nc.dram_tensor("hn_cc", [DQ, S], CDT)
    hnT_ag = nc.dram_tensor("hnT_ag", [D, S], CDT, addr_space="Shared")
    oT_cc = nc.dram_tensor("oT_cc", [D, S], CDT)
    o_rs = nc.dram_tensor("o_rs", [DQ, S], CDT)

    RG = [list(range(CORES))]
    ADD = mybir.AluOpType.add
    BYP = mybir.AluOpType.bypass
    EXP = mybir.ActivationFunctionType.Exp
    SQRT = mybir.ActivationFunctionType.Sqrt
    SILU = mybir.ActivationFunctionType.Silu
    ISQ = 1.0 / math.sqrt(HD)

    with tile.TileContext(nc) as tc:
        with (
            tc.tile_pool(name="persist", bufs=1) as persist,
            tc.tile_pool(name="ps_small", bufs=2, space="PSUM") as ps_small,
        ):
            ones = persist.tile([P, 1], CDT)
            nc.vector.memset(ones[:], 1.0)
            eps_sb = persist.tile([P, 1], f32)
            nc.vector.memset(eps_sb[:], EPS)
            dmask_sb = persist.tile([P, P], f32)
            nc.sync.dma_start(out=dmask_sb[:], in_=dmask[:])
            # h^T shard (fp32) persists to the end
            hT = [persist.tile([P, S], f32, tag=f"hT{i}", name=f"hT{i}")
                  for i in range(4)]

            # ============ stage 0: attn RMSNorm + AllGather ============
            with tc.tile_pool(name="st0", bufs=1) as st0:
                xt = []
                for i in range(4):
                    t = st0.tile([P, S], f32, tag=f"xt{i}")
                    nc.sync.dma_start(out=t[:], in_=xT_s[P * i:P * (i + 1), :])
                    xt.append(t)
                sq = []
                for i in range(4):
                    t = st0.tile([P, S], CDT, tag=f"sq{i}")
                    nc.vector.tensor_mul(t[:], xt[i][:], xt[i][:])
                    sq.append(t)
                ssq_sb = st0.tile([1, S], f32)
                for c in range(4):
                    pt = ps_small.tile([1, 512], f32, tag="one512")
                    for i in range(4):
                        nc.tensor.matmul(pt[:], ones[:], sq[i][:, 512 * c:512 * (c + 1)],
                                         start=(i == 0), stop=(i == 3))
                    nc.any.tensor_copy(out=ssq_sb[:, 512 * c:512 * (c + 1)], in_=pt[:])
                nc.sync.dma_start(out=ssq1_in[:], in_=ssq_sb[:])
                nc.gpsimd.collective_compute(
                    "AllReduce", ADD, ins=[ssq1_in[:]], outs=[ssq1_out[:]],
                    replica_groups=RG)
                s_rep = st0.tile([P, S], f32)
                nc.sync.dma_start(out=s_rep[:], in_=ssq1_out[:].to_broadcast((P, S)))
                nc.scalar.activation(out=s_rep[:], in_=s_rep[:], func=SQRT,
                                     bias=eps_sb[:], scale=1.0 / D)
                nc.vector.reciprocal(out=s_rep[:], in_=s_rep[:])
                for i in range(4):
                    z = st0.tile([P, S], CDT, tag=f"z{i}")
                    nc.vector.tensor_mul(z[:], xt[i][:], s_rep[:])
                    nc.sync.dma_start(out=zs_cc[P * i:P * (i + 1), :], in_=z[:])
                nc.gpsimd.collective_compute(
                    "AllGather", BYP, ins=[zs_cc[:]], outs=[zT_ag[:]],
                    replica_groups=RG)

            if DEBUG:
                nc.sync.dma_start(out=dbg["dbg_zT"][:], in_=zT_ag[:])

            with tc.tile_pool(name="attn_persist", bufs=1) as apst:
                # attnT accumulated across heads, consumed by wo
                attnT = apst.tile([P, NHC, S], CDT)
                # V (token-major) persists from projection into attention
                v_sb = apst.tile([P, ST, DQ], CDT)

                # ========= stage 1: Q/K/V projections (+RoPE on Q,K) =====
                zt_view = zT_ag[:].rearrange("(kt p) s -> p kt s", p=P)
                with (
                    tc.tile_pool(name="st1", bufs=2) as st1,
                    tc.tile_pool(name="st1z", bufs=1) as st1z,
                    tc.tile_pool(name="rope", bufs=2) as rope,
                    tc.tile_pool(name="ps_qkv", bufs=2, space="PSUM") as ps_qkv,
                    tc.tile_pool(name="ps_v", bufs=1, space="PSUM") as ps_v,
                ):
                    cos_sb = st1z.tile([P, S], CDT, tag="cos")
                    sin_sb = st1z.tile([P, S], CDT, tag="sin")
                    nc.sync.dma_start(out=cos_sb[:], in_=cos2[:])
                    nc.sync.dma_start(out=sin_sb[:], in_=sinsg2[:])
                    for cp in range(2):  # 1024-token column blocks
                        cols = slice(1024 * cp, 1024 * (cp + 1))
                        zt = st1z.tile([P, DT, 1024], CDT, tag="zt")
                        nc.sync.dma_start(out=zt[:], in_=zt_view[:, :, cols])
                        # --- Q and K (lhsT = weight tile, rhs = z^T) ---
                        for ot in range(8):
                            wt = st1.tile([P, DT, P], CDT, tag="wqk")
                            nc.sync.dma_start(out=wt[:], in_=w_qk[ot])
                            for cc in range(2):
                                ch = slice(512 * cc, 512 * (cc + 1))
                                gch = slice(1024 * cp + 512 * cc,
                                            1024 * cp + 512 * (cc + 1))
                                pt = ps_qkv.tile([P, 512], f32, tag="pqk")
                                for kt in range(DT):
                                    nc.tensor.matmul(
                                        pt[:], wt[:, kt], zt[:, kt, ch],
                                        start=(kt == 0), stop=(kt == DT - 1))
                                # RoPE (head dims are [evens | odds]):
                                #  out = pt*[c;c] + swap(pt)*[-s;s]
                                swp = rope.tile([P, 512], f32, tag="swp")
                                nc.vector.tensor_copy(swp[0:64, :], pt[64:128, :])
                                nc.vector.tensor_copy(swp[64:128, :], pt[0:64, :])
                                t1 = rope.tile([P, 512], f32, tag="t1")
                                t2 = rope.tile([P, 512], f32, tag="t2")
                                nc.vector.tensor_mul(t1[:], pt[:], cos_sb[:, gch])
                                nc.vector.tensor_mul(t2[:], swp[:], sin_sb[:, gch])
                                qk = rope.tile([P, 512], CDT, tag="qk")
                                nc.vector.tensor_add(qk[:], t1[:], t2[:])
                                dst = qt_dram if ot < 4 else kt_dram
                                h = ot % 4
                                nc.sync.dma_start(
                                    out=dst[P * h:P * (h + 1), gch], in_=qk[:])
                        # --- V (lhsT = z^T token tile, rhs = weight) ---
                        for g in range(2):
                            pts = [ps_v.tile([P, DQ], f32, tag=f"pv{i}", name=f"pv{i}")
                                   for i in range(4)]
                            for kt in range(DT):
                                wv = st1.tile([P, DQ], CDT, tag="wv")
                                nc.sync.dma_start(out=wv[:], in_=w_v[kt])
                                for i in range(4):
                                    tok = slice(P * (4 * g + i), P * (4 * g + i + 1))
                                    nc.tensor.matmul(
                                        pts[i][:], zt[:, kt, tok], wv[:],
                                        start=(kt == 0), stop=(kt == DT - 1))
                            for i in range(4):
                                st = 8 * cp + 4 * g + i
                                nc.any.tensor_copy(out=v_sb[:, st, :], in_=pts[i][:])

                if DEBUG:
                    nc.sync.dma_start(out=dbg["dbg_qt"][:], in_=qt_dram[:])
                    nc.sync.dma_start(out=dbg["dbg_kt"][:], in_=kt_dram[:])
                    nc.sync.dma_start(out=dbg["dbg_v"][:], in_=v_sb[:])

                # ============ stage 2: attention ============
                with (
                    tc.tile_pool(name="st2", bufs=2) as st2,
                    tc.tile_pool(name="exps", bufs=6) as exps,
                    tc.tile_pool(name="ps_sc", bufs=3, space="PSUM") as ps_sc,
                    tc.tile_pool(name="ps_av", bufs=2, space="PSUM") as ps_av,
                ):
                    for h in range(NHC):
                        qt = st2.tile([P, S], CDT, tag="qt")
                        kt_t = st2.tile([P, S], CDT, tag="kt")
                        nc.sync.dma_start(out=qt[:], in_=qt_dram[P * h:P * (h + 1), :])
                        nc.sync.dma_start(out=kt_t[:],
                                          in_=kt_dram[P * h:P * (h + 1), :])
                        for qc in range(NCH):
                            nkt = 4 * qc + 4
                            avp = ps_av.tile([P, 512], f32, tag="avp")
                            smp = ps_small.tile([1, 512], f32, tag="one512")
                            for ktile in range(nkt):
                                diag = ktile >= 4 * qc
                                col0 = 128 * (ktile - 4 * qc) if diag else 0
                                scp = ps_sc.tile([P, 512], f32, tag="scp")
                                nc.tensor.matmul(
                                    scp[:, col0:],
                                    kt_t[:, P * ktile:P * (ktile + 1)],
                                    qt[:, 512 * qc + col0:512 * (qc + 1)],
                                    start=True, stop=True)
                                if diag:
                                    nc.vector.tensor_add(
                                        scp[:, col0:col0 + P],
                                        scp[:, col0:col0 + P], dmask_sb[:])
                                et = exps.tile([P, 512], CDT, tag="et")
                                if col0 > 0:
                                    nc.vector.memset(et[:, 0:col0], 0.0)
                                nc.scalar.activation(out=et[:, col0:],
                                                     in_=scp[:, col0:],
                                                     func=EXP, scale=ISQ)
                                nc.tensor.matmul(
                                    avp[:], v_sb[:, ktile, P * h:P * (h + 1)],
                                    et[:], start=(ktile == 0),
                                    stop=(ktile == nkt - 1))
                                nc.tensor.matmul(smp[:], ones[:], et[:],
                                                 start=(ktile == 0),
                                                 stop=(ktile == nkt - 1))
                            rec = st2.tile([1, 512], f32, tag="rec")
                            nc.vector.reciprocal(out=rec[:], in_=smp[:])
                            slot = 4 * h + qc
                            nc.sync.dma_start(out=sums_dram[slot:slot + 1, :],
                                              in_=rec[:])
                            rrep = st2.tile([P, 512], f32, tag="rrep")
                            nc.sync.dma_start(
                                out=rrep[:],
                                in_=sums_dram[slot:slot + 1, :].to_broadcast((P, 512)))
                            nc.vector.tensor_mul(
                                attnT[:, h, 512 * qc:512 * (qc + 1)], avp[:], rrep[:])

                if DEBUG:
                    nc.sync.dma_start(out=dbg["dbg_attnT"][:], in_=attnT[:])

                # ========= stage 3: wo projection -> ReduceScatter =========
                with (
                    tc.tile_pool(name="st3", bufs=2) as st3,
                    tc.tile_pool(name="st3w", bufs=1) as st3w,
                    tc.tile_pool(name="ps_wo", bufs=2, space="PSUM") as ps_wo,
                ):
                    wo_sb = st3w.tile([P, 32, 4, P], CDT)
                    nc.sync.dma_start(out=wo_sb[:], in_=w_o[:])
                    for ot in range(32):
                        for c in range(NCH):
                            ch = slice(512 * c, 512 * (c + 1))
                            pt = ps_wo.tile([P, 512], f32, tag="pwo")
                            for dt_i in range(4):
                                nc.tensor.matmul(pt[:], wo_sb[:, ot, dt_i],
                                                 attnT[:, dt_i, ch],
                                                 start=(dt_i == 0), stop=(dt_i == 3))
                            yt = st3.tile([P, 512], CDT, tag="yt")
                            nc.any.tensor_copy(out=yt[:], in_=pt[:])
                            nc.sync.dma_start(out=yT_cc[P * ot:P * (ot + 1), ch],
                                              in_=yt[:])
                    nc.gpsimd.collective_compute(
                        "ReduceScatter", ADD, ins=[yT_cc[:]], outs=[y_rs[:]],
                        replica_groups=RG)

            if DEBUG:
                nc.sync.dma_start(out=dbg["dbg_yT"][:], in_=yT_cc[:])
                nc.sync.dma_start(out=dbg["dbg_yrs"][:], in_=y_rs[:])

            # ========= stage 4: residual + FFN RMSNorm + AllGather ========
            with tc.tile_pool(name="st4", bufs=1) as st4:
                for i in range(4):
                    xt_i = st4.tile([P, S], f32, tag=f"x4{i}")
                    nc.sync.dma_start(out=xt_i[:], in_=xT_s[P * i:P * (i + 1), :])
                    ys = st4.tile([P, S], CDT, tag=f"ys{i}")
                    nc.sync.dma_start(out=ys[:], in_=y_rs[P * i:P * (i + 1), :])
                    nc.vector.tensor_add(hT[i][:], xt_i[:], ys[:])
                sq2 = []
                for i in range(4):
                    t = st4.tile([P, S], CDT, tag=f"sq2_{i}")
                    nc.vector.tensor_mul(t[:], hT[i][:], hT[i][:])
                    sq2.append(t)
                ssq_sb2 = st4.tile([1, S], f32)
                for c in range(4):
                    pt = ps_small.tile([1, 512], f32, tag="one512")
                    for i in range(4):
                        nc.tensor.matmul(pt[:], ones[:],
                                         sq2[i][:, 512 * c:512 * (c + 1)],
                                         start=(i == 0), stop=(i == 3))
                    nc.any.tensor_copy(out=ssq_sb2[:, 512 * c:512 * (c + 1)],
                                       in_=pt[:])
                nc.sync.dma_start(out=ssq2_in[:], in_=ssq_sb2[:])
                nc.gpsimd.collective_compute(
                    "AllReduce", ADD, ins=[ssq2_in[:]], outs=[ssq2_out[:]],
                    replica_groups=RG)
                s2_rep = st4.tile([P, S], f32)
                nc.sync.dma_start(out=s2_rep[:], in_=ssq2_out[:].to_broadcast((P, S)))
                nc.scalar.activation(out=s2_rep[:], in_=s2_rep[:], func=SQRT,
                                     bias=eps_sb[:], scale=1.0 / D)
                nc.vector.reciprocal(out=s2_rep[:], in_=s2_rep[:])
                for i in range(4):
                    hn = st4.tile([P, S], CDT, tag=f"hn{i}")
                    nc.vector.tensor_mul(hn[:], hT[i][:], s2_rep[:])
                    nc.sync.dma_start(out=hn_cc[P * i:P * (i + 1), :], in_=hn[:])
                nc.gpsimd.collective_compute(
                    "AllGather", BYP, ins=[hn_cc[:]], outs=[hnT_ag[:]],
                    replica_groups=RG)

            if DEBUG:
                for i in range(4):
                    nc.sync.dma_start(out=dbg["dbg_h"][P * i:P * (i + 1), :],
                                      in_=hT[i][:])
                nc.sync.dma_start(out=dbg["dbg_hnT"][:], in_=hnT_ag[:])

            # ============ stage 5: FFN ============
            hn_view = hnT_ag[:].rearrange("(kt p) s -> p kt s", p=P)
            with (
                tc.tile_pool(name="st5w", bufs=2) as st5w,
                tc.tile_pool(name="st5h", bufs=1) as st5h,
                tc.tile_pool(name="st5g", bufs=1) as st5g,
                tc.tile_pool(name="st5t", bufs=3) as st5t,
                tc.tile_pool(name="ps_f1", bufs=2, space="PSUM") as ps_f1,
                tc.tile_pool(name="ps_f3", bufs=2, space="PSUM") as ps_f3,
                tc.tile_pool(name="ps_w2", bufs=2, space="PSUM") as ps_w2,
            ):
                for cp in range(2):
                    cols = slice(1024 * cp, 1024 * (cp + 1))
                    hn_sb = st5h.tile([P, DT, 1024], CDT, tag="hn")
                    nc.sync.dma_start(out=hn_sb[:], in_=hn_view[:, :, cols])
                    g_sb = st5g.tile([P, FT, 1024], CDT, tag="g")
                    for ft in range(FT):
                        w1t = st5w.tile([P, DT, P], CDT, tag="w1")
                        w3t = st5w.tile([P, DT, P], CDT, tag="w3")
                        nc.sync.dma_start(out=w1t[:], in_=w_1[ft])
                        nc.sync.dma_start(out=w3t[:], in_=w_3[ft])
                        for cc in range(2):
                            ch = slice(512 * cc, 512 * (cc + 1))
                            p1 = ps_f1.tile([P, 512], f32, tag="p1")
                            p3 = ps_f3.tile([P, 512], f32, tag="p3")
                            for kt in range(DT):
                                nc.tensor.matmul(p1[:], w1t[:, kt], hn_sb[:, kt, ch],
                                                 start=(kt == 0), stop=(kt == DT - 1))
                            for kt in range(DT):
                                nc.tensor.matmul(p3[:], w3t[:, kt], hn_sb[:, kt, ch],
                                                 start=(kt == 0), stop=(kt == DT - 1))
                            tsi = st5t.tile([P, 512], CDT, tag="tsi")
                            nc.scalar.activation(out=tsi[:], in_=p1[:], func=SILU)
                            nc.vector.tensor_mul(g_sb[:, ft, ch], tsi[:], p3[:])
                    for ot in range(32):
                        w2t = st5w.tile([P, FT, P], CDT, tag="w2")
                        nc.sync.dma_start(out=w2t[:], in_=w_2[ot])
                        for cc in range(2):
                            ch = slice(512 * cc, 512 * (cc + 1))
                            gch = slice(1024 * cp + 512 * cc,
                                        1024 * cp + 512 * (cc + 1))
                            pt = ps_w2.tile([P, 512], f32, tag="pw2")
                            for ft in range(FT):
                                nc.tensor.matmul(pt[:], w2t[:, ft], g_sb[:, ft, ch],
                                                 start=(ft == 0), stop=(ft == FT - 1))
                            og = st5t.tile([P, 512], CDT, tag="og")
                            nc.any.tensor_copy(out=og[:], in_=pt[:])
                            nc.sync.dma_start(out=oT_cc[P * ot:P * (ot + 1), gch],
                                              in_=og[:])
                nc.gpsimd.collective_compute(
                    "ReduceScatter", ADD, ins=[oT_cc[:]], outs=[o_rs[:]],
                    replica_groups=RG)

            if DEBUG:
                nc.sync.dma_start(out=dbg["dbg_oT"][:], in_=oT_cc[:])
                nc.sync.dma_start(out=dbg["dbg_ors"][:], in_=o_rs[:])

            # ============ stage 6: final residual ============
            with tc.tile_pool(name="st6", bufs=2) as st6:
                for i in range(4):
                    o_sb = st6.tile([P, S], CDT, tag="osb")
                    nc.sync.dma_start(out=o_sb[:], in_=o_rs[P * i:P * (i + 1), :])
                    out_sb = st6.tile([P, S], f32, tag="outsb")
                    nc.vector.tensor_add(out_sb[:], hT[i][:], o_sb[:])
                    nc.sync.dma_start(out=outT_s[P * i:P * (i + 1), :], in_=out_sb[:])

    nc.compile()
    return nc


def _prep_inputs(x, freqs_cos, freqs_sin, mask, attn_norm_w, wq, wk, wv, wo,
                 ffn_norm_w, w1, w2, w3):
    """Host-side sharding + weight layout. Returns in_maps for 8 cores."""
    f32 = np.float32
    x2 = np.asarray(x, f32)[0]                     # [S, D]
    xT = np.ascontiguousarray(x2.T)                # [D, S]
    anw = np.asarray(attn_norm_w, f32)
    fnw = np.asarray(ffn_norm_w, f32)
    wq = np.asarray(wq, f32) * anw[None, :]
    wk = np.asarray(wk, f32) * anw[None, :]
    wv_e = np.asarray(wv, f32)
    wo = np.asarray(wo, f32)
    w1 = np.asarray(w1, f32) * fnw[None, :]
    w3 = np.asarray(w3, f32) * fnw[None, :]
    w2 = np.asarray(w2, f32)

    perm = np.concatenate([np.arange(0, HD, 2), np.arange(1, HD, 2)])

    cosT = np.ascontiguousarray(np.asarray(freqs_cos, f32).T)   # [64, S]
    sinT = np.ascontiguousarray(np.asarray(freqs_sin, f32).T)
    cos2 = np.concatenate([cosT, cosT], axis=0).astype(NP_CDT)  # [128, S]
    sinsg2 = np.concatenate([-sinT, sinT], axis=0).astype(NP_CDT)
    m = np.asarray(mask, f32)[0, 0]
    dmask = (np.ascontiguousarray(m[:P, :P].T) * f32(math.sqrt(HD))).astype(f32)

    def lhsT_tiles(wt, n_out_tiles, n_k_tiles):
        # wt: [K, Mout] -> [ot, p, kt, j] with [ot,p,kt,j] = wt[128*kt+p, 128*ot+j]
        a = wt.reshape(n_k_tiles, P, n_out_tiles, P)
        return np.ascontiguousarray(a.transpose(2, 1, 0, 3)).astype(NP_CDT)

    in_maps = []
    for r in range(CORES):
        ds = slice(DQ * r, DQ * (r + 1))
        # Q, K column shards (+ even/odd perm inside each head), transposed
        wqT = wq[ds].T.copy()                      # [D, DQ]
        wkT = wk[ds].T.copy()
        for h in range(NHC):
            blk = slice(HD * h, HD * (h + 1))
            wqT[:, blk] = wqT[:, blk][:, perm]
            wkT[:, blk] = wkT[:, blk][:, perm]
        wqk = np.concatenate([lhsT_tiles(wqT, NHC, DT),
                              lhsT_tiles(wkT, NHC, DT)], axis=0)  # [8,P,DT,P]
        # V: rhs layout [kt, p, DQ]
        wvT = wv_e[ds].T.copy()                    # [D, DQ]
        w_v_l = np.ascontiguousarray(wvT.reshape(DT, P, DQ)).astype(NP_CDT)
        # wo row shard, transposed -> lhsT tiles -> [p, ot, dt, j]
        woT = wo[:, ds].T.copy()                   # [DQ, D]
        wo_l = lhsT_tiles(woT, 32, 4)              # [32, P, 4, P]
        wo_l = np.ascontiguousarray(wo_l.transpose(1, 0, 2, 3))  # [P,32,4,P]
        # FFN shards (padded to FP rows)
        fs = slice(FC * r, FC * (r + 1))
        w1s = np.zeros((FP, D), f32)
        w3s = np.zeros((FP, D), f32)
        w1s[:FC] = w1[fs]
        w3s[:FC] = w3[fs]
        w1_l = lhsT_tiles(np.ascontiguousarray(w1s.T), FT, DT)  # [FT, P, DT, P]
        w3_l = lhsT_tiles(np.ascontiguousarray(w3s.T), FT, DT)
        w2s = np.zeros((FP, D), f32)
        w2s[:FC] = w2[:, fs].T                     # [FP, D] (rows = f)
        w2_l = lhsT_tiles(w2s, 32, FT)             # [32, P, FT, P]

        in_maps.append({
            "xT_s": np.ascontiguousarray(xT[ds]),
            "w_qk": wqk,
            "w_v": w_v_l,
            "w_o": wo_l,
            "w_1": w1_l,
            "w_3": w3_l,
            "w_2": w2_l,
            "cos2": cos2,
            "sinsg2": sinsg2,
            "dmask": dmask,
        })
    return in_maps


def kernel(x, freqs_cos, freqs_sin, mask, attn_norm_w, wq, wk, wv, wo,
           ffn_norm_w, w1, w2, w3, _trace=False):
    global _COMPILED
    if _COMPILED is None:
        _COMPILED = _build()
    nc = _COMPILED
    in_maps = _prep_inputs(x, freqs_cos, freqs_sin, mask, attn_norm_w,
                           wq, wk, wv, wo, ffn_norm_w, w1, w2, w3)
    res = run_bass_kernel_spmd(nc, in_maps, list(range(CORES)), trace=_trace)
    kernel.last_result = res
    outT = np.concatenate([res.results[r]["outT_s"] for r in range(CORES)],
                          axis=0)                  # [D, S]
    return np.ascontiguousarray(outT.T)[None].astype(np.float32)


# revision 10
# speedup vs baseline: 1.0721x; 1.0721x over previous
"""Llama-style transformer block on 8 TRN2 NeuronCores.

Megatron tensor-parallel with feature-major (transposed) activations:
  - Residual stream kept TRANSPOSED (x^T: [D, S]) so every matmul contracts
    over the partition dim with zero on-chip transposes.
  - Per core: 4 attention heads (512 of 4096 q/k/v dims) and 1376 (padded
    to 1408) of the 11008 FFN hidden dims.
  - RMSNorm: per-core partial sum-of-squares over the 512-feature shard,
    AllReduce [1,2048], scale own shard, AllGather normalized activations
    (feature-stacked = the exact layout the matmuls consume).
  - Attention: transposed scores ([s_k, s_q]) feed the AV matmul directly;
    softmax skips max-subtraction (exact; scores bounded).  Denominators
    via ones-matmul; normalization deferred to attention output.
  - RoPE via host-side even/odd permutation of head dims + two aligned
    half-tile swaps against [cos;cos] / [-sin;sin] tables.
  - wo / w2 partial sums -> token-chunked ReduceScatter, pipelined under
    the next chunk's compute.  AllGathers similarly chunked.

v2: everything pipelined in 512-token chunks to keep TensorE continuously
busy (HAM clock stays warm) and hide collectives under compute.
"""

import math

import ml_dtypes
import numpy as np

import concourse.bass as bass
import concourse.mybir as mybir
import concourse.tile as tile
from concourse import bacc
from concourse.bass_utils import run_bass_kernel_spmd

S = 2048
D = 4096
HD = 128
NH = 32
F = 11008
CORES = 8
NHC = NH // CORES          # heads per core = 4
DQ = NHC * HD              # q/k/v dims per core = 512
FC = F // CORES            # ffn dims per core = 1376
FT = 11                    # padded f-tiles per core
FP = FT * 128
EPS = 1e-5
P = 128
NCH = 4                    # 512-token chunks
CW = S // NCH              # chunk width = 512
DT = D // P                # d tiles = 32
ST = S // P                # s tiles = 16

CDT = mybir.dt.bfloat16
NP_CDT = ml_dtypes.bfloat16

_COMPILED = None


def _build():
    nc = bacc.Bacc("TRN2", target_bir_lowering=False, debug=False,
                   num_devices=CORES)
    f32 = mybir.dt.float32

    # ---- kernel I/O ----
    xT_s = nc.declare_dram_parameter("xT_s", [DQ, S], f32, isOutput=False)
    w_qk = nc.declare_dram_parameter("w_qk", [8, P, DT, P], CDT, isOutput=False)
    w_v = nc.declare_dram_parameter("w_v", [DT, P, DQ], CDT, isOutput=False)
    w_o = nc.declare_dram_parameter("w_o", [P, 32, 4, P], CDT, isOutput=False)
    w_1 = nc.declare_dram_parameter("w_1", [FT, P, DT, P], CDT, isOutput=False)
    w_3 = nc.declare_dram_parameter("w_3", [FT, P, DT, P], CDT, isOutput=False)
    w_2 = nc.declare_dram_parameter("w_2", [32, P, FT, P], CDT, isOutput=False)
    cos2 = nc.declare_dram_parameter("cos2", [P, S], CDT, isOutput=False)
    sinsg2 = nc.declare_dram_parameter("sinsg2", [P, S], CDT, isOutput=False)
    dmask = nc.declare_dram_parameter("dmask", [P, P], f32, isOutput=False)
    outT_s = nc.declare_dram_parameter("outT_s", [DQ, S], f32, isOutput=True)

    # ---- internal DRAM ----
    ssq1_in = nc.dram_tensor("ssq1_in", [1, S], f32)
    ssq1_out = nc.dram_tensor("ssq1_out", [1, S], f32, addr_space="Shared")
    zs_cc = nc.dram_tensor("zs_cc", [DQ, S], CDT)
    zT_ag = nc.dram_tensor("zT_ag", [D, S], CDT, addr_space="Shared")
    qt_dram = nc.dram_tensor("qt_dram", [DQ, S], CDT)
    kt_dram = nc.dram_tensor("kt_dram", [DQ, S], CDT)
    sums_dram = nc.dram_tensor("sums_dram", [16, CW], f32)
    yT_cc = [nc.dram_tensor(f"yT_cc{c}", [D, CW], CDT) for c in range(NCH)]
    y_rs = [nc.dram_tensor(f"y_rs{c}", [DQ, CW], CDT) for c in range(NCH)]
    ssq2_in = nc.dram_tensor("ssq2_in", [1, S], f32)
    ssq2_out = nc.dram_tensor("ssq2_out", [1, S], f32, addr_space="Shared")
    hn_cc = [nc.dram_tensor(f"hn_cc{c}", [DQ, CW], CDT) for c in range(NCH)]
    hnT_ag = [nc.dram_tensor(f"hnT_ag{c}", [D, CW], CDT, addr_space="Shared")
              for c in range(NCH)]
    oT_cc = [nc.dram_tensor(f"oT_cc{c}", [D, CW], CDT) for c in range(NCH)]
    o_rs = [nc.dram_tensor(f"o_rs{c}", [DQ, CW], CDT) for c in range(NCH)]

    RG = [list(range(CORES))]
    ADD = mybir.AluOpType.add
    BYP = mybir.AluOpType.bypass
    EXP = mybir.ActivationFunctionType.Exp
    SQRT = mybir.ActivationFunctionType.Sqrt
    SILU = mybir.ActivationFunctionType.Silu
    ISQ = 1.0 / math.sqrt(HD)

    def ch(c):
        return slice(CW * c, CW * (c + 1))

    with tile.TileContext(nc) as tc:
        with (
            tc.tile_pool(name="persist", bufs=1) as persist,
            tc.tile_pool(name="ps_small", bufs=2, space="PSUM") as ps_small,
        ):
            ones = persist.tile([P, 1], CDT)
            nc.vector.memset(ones[:], 1.0)
            eps_sb = persist.tile([P, 1], f32)
            nc.vector.memset(eps_sb[:], EPS)
            dmask_sb = persist.tile([P, P], f32)
            nc.sync.dma_start(out=dmask_sb[:], in_=dmask[:])
            hT = [persist.tile([P, S], f32, tag=f"hT{i}", name=f"hT{i}")
                  for i in range(4)]

            # ============ stage 0: attn RMSNorm + AllGather(z) ============
            with tc.tile_pool(name="st0", bufs=1) as st0:
                xt = []
                for i in range(4):
                    t = st0.tile([P, S], f32, tag=f"xt{i}")
                    nc.sync.dma_start(out=t[:], in_=xT_s[P * i:P * (i + 1), :])
                    xt.append(t)
                sq = []
                for i in range(4):
                    t = st0.tile([P, S], CDT, tag=f"sq{i}")
                    nc.vector.tensor_mul(t[:], xt[i][:], xt[i][:])
                    sq.append(t)
                ssq_sb = st0.tile([1, S], f32)
                for c in range(NCH):
                    pt = ps_small.tile([1, CW], f32, tag="one512")
                    for i in range(4):
                        nc.tensor.matmul(pt[:], ones[:], sq[i][:, ch(c)],
                                         start=(i == 0), stop=(i == 3))
                    nc.any.tensor_copy(out=ssq_sb[:, ch(c)], in_=pt[:])
                nc.sync.dma_start(out=ssq1_in[:], in_=ssq_sb[:])
                nc.gpsimd.collective_compute(
                    "AllReduce", ADD, ins=[ssq1_in[:]], outs=[ssq1_out[:]],
                    replica_groups=RG)
                s_rep = st0.tile([P, S], f32)
                nc.sync.dma_start(out=s_rep[:], in_=ssq1_out[:].to_broadcast((P, S)))
                nc.scalar.activation(out=s_rep[:], in_=s_rep[:], func=SQRT,
                                     bias=eps_sb[:], scale=1.0 / D)
                nc.vector.reciprocal(out=s_rep[:], in_=s_rep[:])
                for i in range(4):
                    z = st0.tile([P, S], CDT, tag=f"z{i}")
                    nc.vector.tensor_mul(z[:], xt[i][:], s_rep[:])
                    nc.sync.dma_start(out=zs_cc[P * i:P * (i + 1), :], in_=z[:])
                nc.gpsimd.collective_compute(
                    "AllGather", BYP, ins=[zs_cc[:]], outs=[zT_ag[:]],
                    replica_groups=RG)

            with tc.tile_pool(name="attn_persist", bufs=1) as apst:
                attnT = apst.tile([P, NHC, S], CDT)
                v_sb = apst.tile([P, ST, DQ], CDT)

                # ===== stage 1: Q/K/V projections (+RoPE), per 512-chunk ====
                zt_view = zT_ag[:].rearrange("(kt p) s -> p kt s", p=P)
                with (
                    tc.tile_pool(name="st1", bufs=1) as st1,
                    tc.tile_pool(name="st1w", bufs=3) as st1w,
                    tc.tile_pool(name="st1z", bufs=2) as st1z,
                    tc.tile_pool(name="rope", bufs=3) as rope,
                    tc.tile_pool(name="ps_qkv", bufs=2, space="PSUM") as ps_qkv,
                    tc.tile_pool(name="ps_v", bufs=1, space="PSUM") as ps_v,
                ):
                    cos_sb = st1.tile([P, S], CDT, tag="cos")
                    sin_sb = st1.tile([P, S], CDT, tag="sin")
                    nc.sync.dma_start(out=cos_sb[:], in_=cos2[:])
                    nc.sync.dma_start(out=sin_sb[:], in_=sinsg2[:])
                    for c in range(NCH):
                        zt = st1z.tile([P, DT, CW], CDT, tag="zt")
                        nc.sync.dma_start(out=zt[:], in_=zt_view[:, :, ch(c)])
                        # --- Q and K ---
                        for ot in range(8):
                            wt = st1w.tile([P, DT, P], CDT, tag="wqk")
                            nc.sync.dma_start(out=wt[:], in_=w_qk[ot])
                            pt = ps_qkv.tile([P, CW], f32, tag="pqk")
                            for kt in range(DT):
                                nc.tensor.matmul(pt[:], wt[:, kt], zt[:, kt, :],
                                                 start=(kt == 0), stop=(kt == DT - 1))
                            # RoPE: out = pt*[c;c] + swap(pt)*[-s;s]
                            swp = rope.tile([P, CW], f32, tag="swp")
                            nc.vector.tensor_copy(swp[0:64, :], pt[64:128, :])
                            nc.vector.tensor_copy(swp[64:128, :], pt[0:64, :])
                            t1 = rope.tile([P, CW], f32, tag="t1")
                            t2 = rope.tile([P, CW], f32, tag="t2")
                            nc.vector.tensor_mul(t1[:], pt[:], cos_sb[:, ch(c)])
                            nc.vector.tensor_mul(t2[:], swp[:], sin_sb[:, ch(c)])
                            qk = rope.tile([P, CW], CDT, tag="qk")
                            nc.vector.tensor_add(qk[:], t1[:], t2[:])
                            dst = qt_dram if ot < 4 else kt_dram
                            hh = ot % 4
                            nc.sync.dma_start(out=dst[P * hh:P * (hh + 1), ch(c)],
                                              in_=qk[:])
                        # --- V: 4 token-tiles of this chunk ---
                        pts = [ps_v.tile([P, DQ], f32, tag=f"pv{i}", name=f"pv{i}")
                               for i in range(4)]
                        for kt in range(DT):
                            wv = st1w.tile([P, DQ], CDT, tag="wv")
                            nc.sync.dma_start(out=wv[:], in_=w_v[kt])
                            for i in range(4):
                                tok = slice(P * i, P * (i + 1))
                                nc.tensor.matmul(
                                    pts[i][:], zt[:, kt, tok], wv[:],
                                    start=(kt == 0), stop=(kt == DT - 1))
                        for i in range(4):
                            nc.any.tensor_copy(out=v_sb[:, 4 * c + i, :],
                                               in_=pts[i][:])

                # ====== stage 2+3: attention + wo + chunked RS(y) ======
                with (
                    tc.tile_pool(name="st2", bufs=2) as st2,
                    tc.tile_pool(name="st2qk", bufs=1) as st2qk,
                    tc.tile_pool(name="exps", bufs=6) as exps,
                    tc.tile_pool(name="ps_sc", bufs=3, space="PSUM") as ps_sc,
                    tc.tile_pool(name="ps_av", bufs=2, space="PSUM") as ps_av,
                    tc.tile_pool(name="ps_wo", bufs=1, space="PSUM") as ps_wo,
                ):
                    wo_sb = st2qk.tile([P, 32, 4, P], CDT)
                    nc.sync.dma_start(out=wo_sb[:], in_=w_o[:])
                    qts, kts = [], []
                    for hh in range(NHC):
                        qt = st2qk.tile([P, S], CDT, tag=f"qt{hh}", name=f"qt{hh}")
                        kt_t = st2qk.tile([P, S], CDT, tag=f"kt{hh}", name=f"kt{hh}")
                        nc.sync.dma_start(out=qt[:],
                                          in_=qt_dram[P * hh:P * (hh + 1), :])
                        nc.sync.dma_start(out=kt_t[:],
                                          in_=kt_dram[P * hh:P * (hh + 1), :])
                        qts.append(qt)
                        kts.append(kt_t)
                    for qc in range(NCH):
                        nkt = 4 * qc + 4
                        for hh in range(NHC):
                            qt, kt_t = qts[hh], kts[hh]
                            avp = ps_av.tile([P, CW], f32, tag="avp")
                            smp = ps_small.tile([1, CW], f32, tag="one512")
                            for ktile in range(nkt):
                                diag = ktile >= 4 * qc
                                col0 = P * (ktile - 4 * qc) if diag else 0
                                scp = ps_sc.tile([P, CW], f32, tag="scp")
                                nc.tensor.matmul(
                                    scp[:, col0:],
                                    kt_t[:, P * ktile:P * (ktile + 1)],
                                    qt[:, CW * qc + col0:CW * (qc + 1)],
                                    start=True, stop=True)
                                if diag:
                                    nc.vector.tensor_add(
                                        scp[:, col0:col0 + P],
                                        scp[:, col0:col0 + P], dmask_sb[:])
                                et = exps.tile([P, CW], CDT, tag="et")
                                if col0 > 0:
                                    nc.vector.memset(et[:, 0:col0], 0.0)
                                nc.scalar.activation(out=et[:, col0:],
                                                     in_=scp[:, col0:],
                                                     func=EXP, scale=ISQ)
                                nc.tensor.matmul(
                                    avp[:], v_sb[:, ktile, P * hh:P * (hh + 1)],
                                    et[:], start=(ktile == 0),
                                    stop=(ktile == nkt - 1))
                                nc.tensor.matmul(smp[:], ones[:], et[:],
                                                 start=(ktile == 0),
                                                 stop=(ktile == nkt - 1))
                            rec = st2.tile([1, CW], f32, tag="rec")
                            nc.vector.reciprocal(out=rec[:], in_=smp[:])
                            slot = 4 * hh + qc
                            nc.sync.dma_start(out=sums_dram[slot:slot + 1, :],
                                              in_=rec[:])
                            rrep = st2.tile([P, CW], f32, tag="rrep")
                            nc.sync.dma_start(
                                out=rrep[:],
                                in_=sums_dram[slot:slot + 1, :].to_broadcast((P, CW)))
                            nc.vector.tensor_mul(attnT[:, hh, ch(qc)], avp[:],
                                                 rrep[:])
                        # ---- wo for this chunk, then RS it ----
                        for ot in range(32):
                            pt = ps_wo.tile([P, CW], f32, tag="pwo")
                            for dt_i in range(4):
                                nc.tensor.matmul(pt[:], wo_sb[:, ot, dt_i],
                                                 attnT[:, dt_i, ch(qc)],
                                                 start=(dt_i == 0), stop=(dt_i == 3))
                            yt = st2.tile([P, CW], CDT, tag="yt")
                            nc.any.tensor_copy(out=yt[:], in_=pt[:])
                            nc.sync.dma_start(out=yT_cc[qc][P * ot:P * (ot + 1), :],
                                              in_=yt[:])
                        nc.gpsimd.collective_compute(
                            "ReduceScatter", ADD, ins=[yT_cc[qc][:]],
                            outs=[y_rs[qc][:]], replica_groups=RG)

            # ====== stage 4: residual + FFN RMSNorm + chunked AG(hn) ======
            with tc.tile_pool(name="st4", bufs=2) as st4:
                ssq_sb2 = persist.tile([1, S], f32)
                for c in range(NCH):
                    sq2 = []
                    for i in range(4):
                        xt_i = st4.tile([P, CW], f32, tag="x4")
                        nc.sync.dma_start(out=xt_i[:],
                                          in_=xT_s[P * i:P * (i + 1), ch(c)])
                        ys = st4.tile([P, CW], CDT, tag="ys")
                        nc.sync.dma_start(out=ys[:],
                                          in_=y_rs[c][P * i:P * (i + 1), :])
                        nc.vector.tensor_add(hT[i][:, ch(c)], xt_i[:], ys[:])
                        t = st4.tile([P, CW], CDT, tag="sq2")
                        nc.vector.tensor_mul(t[:], hT[i][:, ch(c)], hT[i][:, ch(c)])
                        sq2.append(t)
                    pt = ps_small.tile([1, CW], f32, tag="one512")
                    for i in range(4):
                        nc.tensor.matmul(pt[:], ones[:], sq2[i][:],
                                         start=(i == 0), stop=(i == 3))
                    nc.any.tensor_copy(out=ssq_sb2[:, ch(c)], in_=pt[:])
                nc.sync.dma_start(out=ssq2_in[:], in_=ssq_sb2[:])
                nc.gpsimd.collective_compute(
                    "AllReduce", ADD, ins=[ssq2_in[:]], outs=[ssq2_out[:]],
                    replica_groups=RG)
                s2_rep = st4.tile([P, S], f32, tag="s2rep")
                nc.sync.dma_start(out=s2_rep[:], in_=ssq2_out[:].to_broadcast((P, S)))
                nc.scalar.activation(out=s2_rep[:], in_=s2_rep[:], func=SQRT,
                                     bias=eps_sb[:], scale=1.0 / D)
                nc.vector.reciprocal(out=s2_rep[:], in_=s2_rep[:])
                for c in range(NCH):
                    for i in range(4):
                        hn = st4.tile([P, CW], CDT, tag="hn4")
                        nc.vector.tensor_mul(hn[:], hT[i][:, ch(c)],
                                             s2_rep[:, ch(c)])
                        nc.sync.dma_start(out=hn_cc[c][P * i:P * (i + 1), :],
                                          in_=hn[:])
                    nc.gpsimd.collective_compute(
                        "AllGather", BYP, ins=[hn_cc[c][:]], outs=[hnT_ag[c][:]],
                        replica_groups=RG)

            # ============ stage 5: FFN + chunked RS(o) ============
            with (
                tc.tile_pool(name="st5w", bufs=2) as st5w,
                tc.tile_pool(name="st5w2", bufs=3) as st5w2,
                tc.tile_pool(name="st5h", bufs=1) as st5h,
                tc.tile_pool(name="st5g", bufs=1) as st5g,
                tc.tile_pool(name="st5t", bufs=3) as st5t,
                tc.tile_pool(name="ps_f1", bufs=2, space="PSUM") as ps_f1,
                tc.tile_pool(name="ps_f3", bufs=2, space="PSUM") as ps_f3,
                tc.tile_pool(name="ps_w2", bufs=2, space="PSUM") as ps_w2,
            ):
                for cp in range(2):
                    hn_sb = st5h.tile([P, DT, 2 * CW], CDT, tag="hn")
                    for cc in range(2):
                        c = 2 * cp + cc
                        hv = hnT_ag[c][:].rearrange("(kt p) s -> p kt s", p=P)
                        nc.sync.dma_start(
                            out=hn_sb[:, :, CW * cc:CW * (cc + 1)], in_=hv[:])
                    g_sb = st5g.tile([P, FT, 2 * CW], CDT, tag="g")
                    for ft in range(FT):
                        w1t = st5w.tile([P, DT, P], CDT, tag="w1")
                        w3t = st5w.tile([P, DT, P], CDT, tag="w3")
                        nc.sync.dma_start(out=w1t[:], in_=w_1[ft])
                        nc.sync.dma_start(out=w3t[:], in_=w_3[ft])
                        for cc in range(2):
                            cs = slice(CW * cc, CW * (cc + 1))
                            p1 = ps_f1.tile([P, CW], f32, tag="p1")
                            p3 = ps_f3.tile([P, CW], f32, tag="p3")
                            for kt in range(DT):
                                nc.tensor.matmul(p1[:], w1t[:, kt], hn_sb[:, kt, cs],
                                                 start=(kt == 0), stop=(kt == DT - 1))
                            for kt in range(DT):
                                nc.tensor.matmul(p3[:], w3t[:, kt], hn_sb[:, kt, cs],
                                                 start=(kt == 0), stop=(kt == DT - 1))
                            tsi = st5t.tile([P, CW], CDT, tag="tsi")
                            nc.scalar.activation(out=tsi[:], in_=p1[:], func=SILU)
                            nc.vector.tensor_mul(g_sb[:, ft, cs], tsi[:], p3[:])
                    for cc in range(2):
                        c = 2 * cp + cc
                        cs = slice(CW * cc, CW * (cc + 1))
                        for ot in range(32):
                            w2t = st5w2.tile([P, FT, P], CDT, tag="w2")
                            nc.sync.dma_start(out=w2t[:], in_=w_2[ot])
                            pt = ps_w2.tile([P, CW], f32, tag="pw2")
                            for ft in range(FT):
                                nc.tensor.matmul(pt[:], w2t[:, ft], g_sb[:, ft, cs],
                                                 start=(ft == 0), stop=(ft == FT - 1))
                            og = st5t.tile([P, CW], CDT, tag="og")
                            nc.any.tensor_copy(out=og[:], in_=pt[:])
                            nc.sync.dma_start(out=oT_cc[c][P * ot:P * (ot + 1), :],
                                              in_=og[:])
                        nc.gpsimd.collective_compute(
                            "ReduceScatter", ADD, ins=[oT_cc[c][:]],
                            outs=[o_rs[c][:]], replica_groups=RG)

            # ============ stage 6: final residual ============
            with tc.tile_pool(name="st6", bufs=2) as st6:
                for c in range(NCH):
                    for i in range(4):
                        o_sb = st6.tile([P, CW], CDT, tag="osb")
                        nc.sync.dma_start(out=o_sb[:],
                                          in_=o_rs[c][P * i:P * (i + 1), :])
                        out_sb = st6.tile([P, CW], f32, tag="outsb")
                        nc.vector.tensor_add(out_sb[:], hT[i][:, ch(c)], o_sb[:])
                        nc.sync.dma_start(out=outT_s[P * i:P * (i + 1), ch(c)],
                                          in_=out_sb[:])

    nc.compile()
    return nc


def _prep_inputs(x, freqs_cos, freqs_sin, mask, attn_norm_w, wq, wk, wv, wo,
                 ffn_norm_w, w1, w2, w3):
    """Host-side sharding + weight layout. Returns in_maps for 8 cores."""
    f32 = np.float32
    x2 = np.asarray(x, f32)[0]                     # [S, D]
    xT = np.ascontiguousarray(x2.T)                # [D, S]
    anw = np.asarray(attn_norm_w, f32)
    fnw = np.asarray(ffn_norm_w, f32)
    wq = np.asarray(wq, f32) * anw[None, :]
    wk = np.asarray(wk, f32) * anw[None, :]
    wv_e = np.asarray(wv, f32)
    wo = np.asarray(wo, f32)
    w1 = np.asarray(w1, f32) * fnw[None, :]
    w3 = np.asarray(w3, f32) * fnw[None, :]
    w2 = np.asarray(w2, f32)

    perm = np.concatenate([np.arange(0, HD, 2), np.arange(1, HD, 2)])

    cosT = np.ascontiguousarray(np.asarray(freqs_cos, f32).T)   # [64, S]
    sinT = np.ascontiguousarray(np.asarray(freqs_sin, f32).T)
    cos2 = np.concatenate([cosT, cosT], axis=0).astype(NP_CDT)  # [128, S]
    sinsg2 = np.concatenate([-sinT, sinT], axis=0).astype(NP_CDT)
    m = np.asarray(mask, f32)[0, 0]
    dmask = (np.ascontiguousarray(m[:P, :P].T) * f32(math.sqrt(HD))).astype(f32)

    def lhsT_tiles(wt, n_out_tiles, n_k_tiles):
        # wt: [K, Mout] -> [ot, p, kt, j] with [ot,p,kt,j] = wt[128*kt+p, 128*ot+j]
        a = wt.reshape(n_k_tiles, P, n_out_tiles, P)
        return np.ascontiguousarray(a.transpose(2, 1, 0, 3)).astype(NP_CDT)

    in_maps = []
    for r in range(CORES):
        ds = slice(DQ * r, DQ * (r + 1))
        wqT = wq[ds].T.copy()                      # [D, DQ]
        wkT = wk[ds].T.copy()
        for h in range(NHC):
            blk = slice(HD * h, HD * (h + 1))
            wqT[:, blk] = wqT[:, blk][:, perm]
            wkT[:, blk] = wkT[:, blk][:, perm]
        wqk = np.concatenate([lhsT_tiles(wqT, NHC, DT),
                              lhsT_tiles(wkT, NHC, DT)], axis=0)  # [8,P,DT,P]
        wvT = wv_e[ds].T.copy()                    # [D, DQ]
        w_v_l = np.ascontiguousarray(wvT.reshape(DT, P, DQ)).astype(NP_CDT)
        woT = wo[:, ds].T.copy()                   # [DQ, D]
        wo_l = lhsT_tiles(woT, 32, 4)              # [32, P, 4, P]
        wo_l = np.ascontiguousarray(wo_l.transpose(1, 0, 2, 3))  # [P,32,4,P]
        fs = slice(FC * r, FC * (r + 1))
        w1s = np.zeros((FP, D), f32)
        w3s = np.zeros((FP, D), f32)
        w1s[:FC] = w1[fs]
        w3s[:FC] = w3[fs]
        w1_l = lhsT_tiles(np.ascontiguousarray(w1s.T), FT, DT)  # [FT, P, DT, P]
        w3_l = lhsT_tiles(np.ascontiguousarray(w3s.T), FT, DT)
        w2s = np.zeros((FP, D), f32)
        w2s[:FC] = w2[:, fs].T                     # [FP, D] (rows = f)
        w2_l = lhsT_tiles(w2s, 32, FT)             # [32, P, FT, P]

        in_maps.append({
            "xT_s": np.ascontiguousarray(xT[ds]),
            "w_qk": wqk,
            "w_v": w_v_l,
            "w_o": wo_l,
            "w_1": w1_l,
            "w_3": w3_l,
            "w_2": w2_l,
            "cos2": cos2,
            "sinsg2": sinsg2,
            "dmask": dmask,
        })
    return in_maps


def kernel(x, freqs_cos, freqs_sin, mask, attn_norm_w, wq, wk, wv, wo,
           ffn_norm_w, w1, w2, w3, _trace=False):
    global _COMPILED
    if _COMPILED is None:
        _COMPILED = _build()
    nc = _COMPILED
    in_maps = _prep_inputs(x, freqs_cos, freqs_sin, mask, attn_norm_w,
                           wq, wk, wv, wo, ffn_norm_w, w1, w2, w3)
    res = run_bass_kernel_spmd(nc, in_maps, list(range(CORES)), trace=_trace)
    kernel.last_result = res
    outT = np.concatenate([res.results[r]["outT_s"] for r in range(CORES)],
                          axis=0)                  # [D, S]
    return np.ascontiguousarray(outT.T)[None].astype(np.float32)


# revision 12
# speedup vs baseline: 1.1326x; 1.0564x over previous
"""Llama-style transformer block on 8 TRN2 NeuronCores.

Megatron tensor-parallel with feature-major (transposed) activations:
  - Residual stream kept TRANSPOSED (x^T: [D, S]) so every matmul contracts
    over the partition dim with zero on-chip transposes.
  - Per core: 4 attention heads (512 of 4096 q/k/v dims) and 1376 (padded
    to 1408) of the 11008 FFN hidden dims.
  - RMSNorm: per-core partial sum-of-squares over the 512-feature shard,
    AllReduce [1,2048], scale own shard, AllGather normalized activations
    (feature-stacked = the exact layout the matmuls consume).
  - Attention: transposed scores ([s_k, s_q]) feed the AV matmul directly;
    softmax skips max-subtraction (exact; scores bounded).  Denominators
    via ones-matmul; normalization deferred to attention output.
  - RoPE via host-side even/odd permutation of head dims + two aligned
    half-tile swaps against [cos;cos] / [-sin;sin] tables.
  - wo / w2 partial sums -> token-chunked ReduceScatter, pipelined under
    the next chunk's compute.  AllGathers similarly chunked.

v2: everything pipelined in 512-token chunks to keep TensorE continuously
busy (HAM clock stays warm) and hide collectives under compute.
"""

import math

import ml_dtypes
import numpy as np

import concourse.bass as bass
import concourse.mybir as mybir
import concourse.tile as tile
from concourse import bacc
from concourse.bass_utils import run_bass_kernel_spmd

S = 2048
D = 4096
HD = 128
NH = 32
F = 11008
CORES = 8
NHC = NH // CORES          # heads per core = 4
DQ = NHC * HD              # q/k/v dims per core = 512
FC = F // CORES            # ffn dims per core = 1376
FT = 11                    # padded f-tiles per core
FP = FT * 128
EPS = 1e-5
P = 128
NCH = 4                    # 512-token chunks
CW = S // NCH              # chunk width = 512
DT = D // P                # d tiles = 32
ST = S // P                # s tiles = 16

CDT = mybir.dt.bfloat16
NP_CDT = ml_dtypes.bfloat16

_COMPILED = None


def _build():
    nc = bacc.Bacc("TRN2", target_bir_lowering=False, debug=False,
                   num_devices=CORES)
    f32 = mybir.dt.float32

    # ---- kernel I/O ----
    xT_s = nc.declare_dram_parameter("xT_s", [DQ, S], f32, isOutput=False)
    w_qk = nc.declare_dram_parameter("w_qk", [8, P, DT, P], CDT, isOutput=False)
    w_v = nc.declare_dram_parameter("w_v", [DT, P, DQ], CDT, isOutput=False)
    w_o = nc.declare_dram_parameter("w_o", [P, 32, 4, P], CDT, isOutput=False)
    w_1 = nc.declare_dram_parameter("w_1", [FT, P, DT, P], CDT, isOutput=False)
    w_3 = nc.declare_dram_parameter("w_3", [FT, P, DT, P], CDT, isOutput=False)
    w_2 = nc.declare_dram_parameter("w_2", [32, P, FT, P], CDT, isOutput=False)
    cos2 = nc.declare_dram_parameter("cos2", [P, S], CDT, isOutput=False)
    sinsg2 = nc.declare_dram_parameter("sinsg2", [P, S], CDT, isOutput=False)
    dmask = nc.declare_dram_parameter("dmask", [P, P], f32, isOutput=False)
    outT_s = nc.declare_dram_parameter("outT_s", [DQ, S], f32, isOutput=True)

    # ---- internal DRAM ----
    ssq1_in = nc.dram_tensor("ssq1_in", [1, S], f32)
    ssq1_out = nc.dram_tensor("ssq1_out", [1, S], f32, addr_space="Shared")
    zs_cc = [nc.dram_tensor(f"zs_cc{c}", [DQ, CW], CDT) for c in range(NCH)]
    zT_ag = [nc.dram_tensor(f"zT_ag{c}", [D, CW], CDT, addr_space="Shared")
             for c in range(NCH)]
    qt_dram = nc.dram_tensor("qt_dram", [DQ, S], CDT)
    kt_dram = nc.dram_tensor("kt_dram", [DQ, S], CDT)
    sums_dram = nc.dram_tensor("sums_dram", [16, CW], f32)
    yT_cc = [nc.dram_tensor(f"yT_cc{c}", [D, CW], CDT) for c in range(NCH)]
    y_rs = [nc.dram_tensor(f"y_rs{c}", [DQ, CW], CDT) for c in range(NCH)]
    ssq2_in = nc.dram_tensor("ssq2_in", [1, S], f32)
    ssq2_out = nc.dram_tensor("ssq2_out", [1, S], f32, addr_space="Shared")
    hn_cc = [nc.dram_tensor(f"hn_cc{c}", [DQ, CW], CDT) for c in range(NCH)]
    hnT_ag = [nc.dram_tensor(f"hnT_ag{c}", [D, CW], CDT, addr_space="Shared")
              for c in range(NCH)]
    oT_cc = [nc.dram_tensor(f"oT_cc{c}", [D, CW], CDT) for c in range(NCH)]
    o_rs = [nc.dram_tensor(f"o_rs{c}", [DQ, CW], CDT) for c in range(NCH)]

    RG = [list(range(CORES))]
    ADD = mybir.AluOpType.add
    BYP = mybir.AluOpType.bypass
    EXP = mybir.ActivationFunctionType.Exp
    SQRT = mybir.ActivationFunctionType.Sqrt
    SILU = mybir.ActivationFunctionType.Silu
    ISQ = 1.0 / math.sqrt(HD)

    def ch(c):
        return slice(CW * c, CW * (c + 1))

    with tile.TileContext(nc) as tc:
        with (
            tc.tile_pool(name="persist", bufs=1) as persist,
            tc.tile_pool(name="ps_small", bufs=1, space="PSUM") as ps_small,
        ):
            ones = persist.tile([P, 1], CDT)
            nc.vector.memset(ones[:], 1.0)
            eps_sb = persist.tile([P, 1], f32)
            nc.vector.memset(eps_sb[:], EPS)
            dmask_sb = persist.tile([P, P], f32)
            nc.sync.dma_start(out=dmask_sb[:], in_=dmask[:])
            hT = [persist.tile([P, S], f32, tag=f"hT{i}", name=f"hT{i}")
                  for i in range(4)]

            # ============ stage 0: attn RMSNorm + AllGather(z) ============
            with tc.tile_pool(name="st0", bufs=1) as st0:
                xt = []
                for i in range(4):
                    t = st0.tile([P, S], f32, tag=f"xt{i}")
                    nc.sync.dma_start(out=t[:], in_=xT_s[P * i:P * (i + 1), :])
                    xt.append(t)
                sq = []
                for i in range(4):
                    t = st0.tile([P, S], CDT, tag=f"sq{i}")
                    nc.vector.tensor_mul(t[:], xt[i][:], xt[i][:])
                    sq.append(t)
                ssq_sb = st0.tile([1, S], f32)
                for c in range(NCH):
                    pt = ps_small.tile([1, CW], f32, tag="one512")
                    for i in range(4):
                        nc.tensor.matmul(pt[:], ones[:], sq[i][:, ch(c)],
                                         start=(i == 0), stop=(i == 3))
                    nc.any.tensor_copy(out=ssq_sb[:, ch(c)], in_=pt[:])
                nc.sync.dma_start(out=ssq1_in[:], in_=ssq_sb[:])
                nc.gpsimd.collective_compute(
                    "AllReduce", ADD, ins=[ssq1_in[:]], outs=[ssq1_out[:]],
                    replica_groups=RG)
                s_rep = st0.tile([P, S], f32)
                nc.sync.dma_start(out=s_rep[:], in_=ssq1_out[:].to_broadcast((P, S)))
                nc.scalar.activation(out=s_rep[:], in_=s_rep[:], func=SQRT,
                                     bias=eps_sb[:], scale=1.0 / D)
                nc.vector.reciprocal(out=s_rep[:], in_=s_rep[:])
                zl = []
                for i in range(4):
                    z = st0.tile([P, S], CDT, tag=f"z{i}")
                    nc.vector.tensor_mul(z[:], xt[i][:], s_rep[:])
                    zl.append(z)
                for c in range(NCH):
                    for i in range(4):
                        nc.sync.dma_start(out=zs_cc[c][P * i:P * (i + 1), :],
                                          in_=zl[i][:, ch(c)])
                    nc.gpsimd.collective_compute(
                        "AllGather", BYP, ins=[zs_cc[c][:]], outs=[zT_ag[c][:]],
                        replica_groups=RG)

            with tc.tile_pool(name="attn_persist", bufs=1) as apst:
                attnT = apst.tile([P, NHC, S], CDT)
                v_sb = apst.tile([P, ST, DQ], CDT)

                # ===== stage 1: Q/K/V projections (+RoPE), per 512-chunk ====
                with (
                    tc.tile_pool(name="st1", bufs=1) as st1,
                    tc.tile_pool(name="st1w", bufs=3) as st1w,
                    tc.tile_pool(name="st1z", bufs=2) as st1z,
                    tc.tile_pool(name="rope", bufs=3) as rope,
                    tc.tile_pool(name="ps_qkv", bufs=2, space="PSUM") as ps_qkv,
                    tc.tile_pool(name="ps_v", bufs=1, space="PSUM") as ps_v,
                ):
                    cos_sb = st1.tile([P, S], CDT, tag="cos")
                    sin_sb = st1.tile([P, S], CDT, tag="sin")
                    nc.sync.dma_start(out=cos_sb[:], in_=cos2[:])
                    nc.sync.dma_start(out=sin_sb[:], in_=sinsg2[:])
                    for c in range(NCH):
                      with nc.named_scope(f"qkv_c{c}"):
                        zt = st1z.tile([P, DT, CW], CDT, tag="zt")
                        zv = zT_ag[c][:].rearrange("(kt p) s -> p kt s", p=P)
                        nc.sync.dma_start(out=zt[:], in_=zv)
                        # --- Q and K ---
                        for ot in range(8):
                            wt = st1w.tile([P, DT, P], CDT, tag="wqk")
                            nc.sync.dma_start(out=wt[:], in_=w_qk[ot])
                            pt = ps_qkv.tile([P, CW], f32, tag="pqk")
                            for kt in range(DT):
                                nc.tensor.matmul(pt[:], wt[:, kt], zt[:, kt, :],
                                                 start=(kt == 0), stop=(kt == DT - 1))
                            # RoPE: out = pt*[c;c] + swap(pt)*[-s;s]
                            swp = rope.tile([P, CW], f32, tag="swp")
                            nc.vector.tensor_copy(swp[0:64, :], pt[64:128, :])
                            nc.vector.tensor_copy(swp[64:128, :], pt[0:64, :])
                            t1 = rope.tile([P, CW], f32, tag="t1")
                            t2 = rope.tile([P, CW], f32, tag="t2")
                            nc.vector.tensor_mul(t1[:], pt[:], cos_sb[:, ch(c)])
                            nc.vector.tensor_mul(t2[:], swp[:], sin_sb[:, ch(c)])
                            qk = rope.tile([P, CW], CDT, tag="qk")
                            nc.vector.tensor_add(qk[:], t1[:], t2[:])
                            dst = qt_dram if ot < 4 else kt_dram
                            hh = ot % 4
                            nc.sync.dma_start(out=dst[P * hh:P * (hh + 1), ch(c)],
                                              in_=qk[:])
                        # --- V: 4 token-tiles of this chunk ---
                        pts = [ps_v.tile([P, DQ], f32, tag=f"pv{i}", name=f"pv{i}")
                               for i in range(4)]
                        for kt in range(DT):
                            wv = st1w.tile([P, DQ], CDT, tag="wv")
                            nc.sync.dma_start(out=wv[:], in_=w_v[kt])
                            for i in range(4):
                                tok = slice(P * i, P * (i + 1))
                                nc.tensor.matmul(
                                    pts[i][:], zt[:, kt, tok], wv[:],
                                    start=(kt == 0), stop=(kt == DT - 1))
                        for i in range(4):
                            nc.any.tensor_copy(out=v_sb[:, 4 * c + i, :],
                                               in_=pts[i][:])

                # ====== stage 2+3: attention + wo + chunked RS(y) ======
                with (
                    tc.tile_pool(name="st2", bufs=2) as st2,
                    tc.tile_pool(name="st2qk", bufs=1) as st2qk,
                    tc.tile_pool(name="exps", bufs=6) as exps,
                    tc.tile_pool(name="ps_sc", bufs=3, space="PSUM") as ps_sc,
                    tc.tile_pool(name="ps_av", bufs=2, space="PSUM") as ps_av,
                    tc.tile_pool(name="ps_wo", bufs=2, space="PSUM") as ps_wo,
                ):
                    wo_sb = st2qk.tile([P, 32, 4, P], CDT)
                    nc.sync.dma_start(out=wo_sb[:], in_=w_o[:])
                    qts, kts = [], []
                    for hh in range(NHC):
                        qt = st2qk.tile([P, S], CDT, tag=f"qt{hh}", name=f"qt{hh}")
                        kt_t = st2qk.tile([P, S], CDT, tag=f"kt{hh}", name=f"kt{hh}")
                        nc.sync.dma_start(out=qt[:],
                                          in_=qt_dram[P * hh:P * (hh + 1), :])
                        nc.sync.dma_start(out=kt_t[:],
                                          in_=kt_dram[P * hh:P * (hh + 1), :])
                        qts.append(qt)
                        kts.append(kt_t)
                    for qc in range(NCH):
                      with nc.named_scope(f"attn_c{qc}"):
                        nkt = 4 * qc + 4
                        for hh in range(NHC):
                            qt, kt_t = qts[hh], kts[hh]
                            avp = ps_av.tile([P, CW], f32, tag="avp")
                            smp = ps_small.tile([1, CW], f32, tag="one512")
                            for ktile in range(nkt):
                                diag = ktile >= 4 * qc
                                col0 = P * (ktile - 4 * qc) if diag else 0
                                scp = ps_sc.tile([P, CW], f32, tag="scp")
                                nc.tensor.matmul(
                                    scp[:, col0:],
                                    kt_t[:, P * ktile:P * (ktile + 1)],
                                    qt[:, CW * qc + col0:CW * (qc + 1)],
                                    start=True, stop=True)
                                if diag:
                                    nc.vector.tensor_add(
                                        scp[:, col0:col0 + P],
                                        scp[:, col0:col0 + P], dmask_sb[:])
                                et = exps.tile([P, CW], CDT, tag="et")
                                if col0 > 0:
                                    nc.vector.memset(et[:, 0:col0], 0.0)
                                nc.scalar.activation(out=et[:, col0:],
                                                     in_=scp[:, col0:],
                                                     func=EXP, scale=ISQ)
                                nc.tensor.matmul(
                                    avp[:], v_sb[:, ktile, P * hh:P * (hh + 1)],
                                    et[:], start=(ktile == 0),
                                    stop=(ktile == nkt - 1))
                                nc.tensor.matmul(smp[:], ones[:], et[:],
                                                 start=(ktile == 0),
                                                 stop=(ktile == nkt - 1))
                            rec = st2.tile([1, CW], f32, tag="rec")
                            nc.vector.reciprocal(out=rec[:], in_=smp[:])
                            slot = 4 * hh + qc
                            nc.sync.dma_start(out=sums_dram[slot:slot + 1, :],
                                              in_=rec[:])
                            rrep = st2.tile([P, CW], f32, tag="rrep")
                            nc.sync.dma_start(
                                out=rrep[:],
                                in_=sums_dram[slot:slot + 1, :].to_broadcast((P, CW)))
                            nc.vector.tensor_mul(attnT[:, hh, ch(qc)], avp[:],
                                                 rrep[:])
                        # ---- wo for this chunk, then RS it ----
                        for ot in range(32):
                            pt = ps_wo.tile([P, CW], f32, tag="pwo")
                            for dt_i in range(4):
                                nc.tensor.matmul(pt[:], wo_sb[:, ot, dt_i],
                                                 attnT[:, dt_i, ch(qc)],
                                                 start=(dt_i == 0), stop=(dt_i == 3))
                            yt = st2.tile([P, CW], CDT, tag="yt")
                            if ot % 2 == 0:
                                nc.vector.tensor_copy(out=yt[:], in_=pt[:])
                            else:
                                nc.scalar.copy(out=yt[:], in_=pt[:])
                            nc.sync.dma_start(out=yT_cc[qc][P * ot:P * (ot + 1), :],
                                              in_=yt[:])
                        nc.gpsimd.collective_compute(
                            "ReduceScatter", ADD, ins=[yT_cc[qc][:]],
                            outs=[y_rs[qc][:]], replica_groups=RG)

            # ====== stage 4: residual + FFN RMSNorm + chunked AG(hn) ======
            with tc.tile_pool(name="st4", bufs=2) as st4:
                ssq_sb2 = persist.tile([1, S], f32)
                for c in range(NCH):
                    sq2 = []
                    for i in range(4):
                        xt_i = st4.tile([P, CW], f32, tag="x4")
                        nc.sync.dma_start(out=xt_i[:],
                                          in_=xT_s[P * i:P * (i + 1), ch(c)])
                        ys = st4.tile([P, CW], CDT, tag="ys")
                        nc.sync.dma_start(out=ys[:],
                                          in_=y_rs[c][P * i:P * (i + 1), :])
                        nc.vector.tensor_add(hT[i][:, ch(c)], xt_i[:], ys[:])
                        t = st4.tile([P, CW], CDT, tag="sq2")
                        nc.vector.tensor_mul(t[:], hT[i][:, ch(c)], hT[i][:, ch(c)])
                        sq2.append(t)
                    pt = ps_small.tile([1, CW], f32, tag="one512")
                    for i in range(4):
                        nc.tensor.matmul(pt[:], ones[:], sq2[i][:],
                                         start=(i == 0), stop=(i == 3))
                    nc.any.tensor_copy(out=ssq_sb2[:, ch(c)], in_=pt[:])
                nc.sync.dma_start(out=ssq2_in[:], in_=ssq_sb2[:])
                nc.gpsimd.collective_compute(
                    "AllReduce", ADD, ins=[ssq2_in[:]], outs=[ssq2_out[:]],
                    replica_groups=RG)
                s2_rep = st4.tile([P, S], f32, tag="s2rep")
                nc.sync.dma_start(out=s2_rep[:], in_=ssq2_out[:].to_broadcast((P, S)))
                nc.scalar.activation(out=s2_rep[:], in_=s2_rep[:], func=SQRT,
                                     bias=eps_sb[:], scale=1.0 / D)
                nc.vector.reciprocal(out=s2_rep[:], in_=s2_rep[:])
                for c in range(NCH):
                    for i in range(4):
                        hn = st4.tile([P, CW], CDT, tag="hn4")
                        nc.vector.tensor_mul(hn[:], hT[i][:, ch(c)],
                                             s2_rep[:, ch(c)])
                        nc.sync.dma_start(out=hn_cc[c][P * i:P * (i + 1), :],
                                          in_=hn[:])
                    nc.gpsimd.collective_compute(
                        "AllGather", BYP, ins=[hn_cc[c][:]], outs=[hnT_ag[c][:]],
                        replica_groups=RG)

            # ============ stage 5: FFN + chunked RS(o) ============
            with (
                tc.tile_pool(name="st5w", bufs=2) as st5w,
                tc.tile_pool(name="st5w2", bufs=3) as st5w2,
                tc.tile_pool(name="st5h", bufs=2) as st5h,
                tc.tile_pool(name="st5g", bufs=2) as st5g,
                tc.tile_pool(name="st5t", bufs=3) as st5t,
                tc.tile_pool(name="ps_f1", bufs=2, space="PSUM") as ps_f1,
                tc.tile_pool(name="ps_f3", bufs=2, space="PSUM") as ps_f3,
                tc.tile_pool(name="ps_w2", bufs=2, space="PSUM") as ps_w2,
            ):
                for c in range(NCH):
                    with nc.named_scope(f"ffn_c{c}"):
                        hn_sb = st5h.tile([P, DT, CW], CDT, tag="hn")
                        hv = hnT_ag[c][:].rearrange("(kt p) s -> p kt s", p=P)
                        nc.sync.dma_start(out=hn_sb[:], in_=hv)
                        g_sb = st5g.tile([P, FT, CW], CDT, tag="g")
                        for ft in range(FT):
                            w1t = st5w.tile([P, DT, P], CDT, tag="w1")
                            w3t = st5w.tile([P, DT, P], CDT, tag="w3")
                            nc.sync.dma_start(out=w1t[:], in_=w_1[ft])
                            nc.sync.dma_start(out=w3t[:], in_=w_3[ft])
                            p1 = ps_f1.tile([P, CW], f32, tag="p1")
                            p3 = ps_f3.tile([P, CW], f32, tag="p3")
                            for kt in range(DT):
                                nc.tensor.matmul(p1[:], w1t[:, kt], hn_sb[:, kt, :],
                                                 start=(kt == 0), stop=(kt == DT - 1))
                            for kt in range(DT):
                                nc.tensor.matmul(p3[:], w3t[:, kt], hn_sb[:, kt, :],
                                                 start=(kt == 0), stop=(kt == DT - 1))
                            tsi = st5t.tile([P, CW], CDT, tag="tsi")
                            nc.scalar.activation(out=tsi[:], in_=p1[:], func=SILU)
                            nc.vector.tensor_mul(g_sb[:, ft, :], tsi[:], p3[:])
                        for ot in range(32):
                            w2t = st5w2.tile([P, FT, P], CDT, tag="w2")
                            nc.sync.dma_start(out=w2t[:], in_=w_2[ot])
                            pt = ps_w2.tile([P, CW], f32, tag="pw2")
                            for ft in range(FT):
                                nc.tensor.matmul(pt[:], w2t[:, ft], g_sb[:, ft, :],
                                                 start=(ft == 0), stop=(ft == FT - 1))
                            og = st5t.tile([P, CW], CDT, tag="og")
                            if ot % 2 == 0:
                                nc.vector.tensor_copy(out=og[:], in_=pt[:])
                            else:
                                nc.scalar.copy(out=og[:], in_=pt[:])
                            nc.sync.dma_start(out=oT_cc[c][P * ot:P * (ot + 1), :],
                                              in_=og[:])
                        nc.gpsimd.collective_compute(
                            "ReduceScatter", ADD, ins=[oT_cc[c][:]],
                            outs=[o_rs[c][:]], replica_groups=RG)

            # ============ stage 6: final residual ============
            with tc.tile_pool(name="st6", bufs=2) as st6:
                for c in range(NCH):
                    for i in range(4):
                        o_sb = st6.tile([P, CW], CDT, tag="osb")
                        nc.sync.dma_start(out=o_sb[:],
                                          in_=o_rs[c][P * i:P * (i + 1), :])
                        out_sb = st6.tile([P, CW], f32, tag="outsb")
                        nc.vector.tensor_add(out_sb[:], hT[i][:, ch(c)], o_sb[:])
                        nc.sync.dma_start(out=outT_s[P * i:P * (i + 1), ch(c)],
                                          in_=out_sb[:])

    nc.compile()
    return nc


def _prep_inputs(x, freqs_cos, freqs_sin, mask, attn_norm_w, wq, wk, wv, wo,
                 ffn_norm_w, w1, w2, w3):
    """Host-side sharding + weight layout. Returns in_maps for 8 cores."""
    f32 = np.float32
    x2 = np.asarray(x, f32)[0]                     # [S, D]
    xT = np.ascontiguousarray(x2.T)                # [D, S]
    anw = np.asarray(attn_norm_w, f32)
    fnw = np.asarray(ffn_norm_w, f32)
    wq = np.asarray(wq, f32) * anw[None, :]
    wk = np.asarray(wk, f32) * anw[None, :]
    wv_e = np.asarray(wv, f32)
    wo = np.asarray(wo, f32)
    w1 = np.asarray(w1, f32) * fnw[None, :]
    w3 = np.asarray(w3, f32) * fnw[None, :]
    w2 = np.asarray(w2, f32)

    perm = np.concatenate([np.arange(0, HD, 2), np.arange(1, HD, 2)])

    cosT = np.ascontiguousarray(np.asarray(freqs_cos, f32).T)   # [64, S]
    sinT = np.ascontiguousarray(np.asarray(freqs_sin, f32).T)
    cos2 = np.concatenate([cosT, cosT], axis=0).astype(NP_CDT)  # [128, S]
    sinsg2 = np.concatenate([-sinT, sinT], axis=0).astype(NP_CDT)
    m = np.asarray(mask, f32)[0, 0]
    dmask = (np.ascontiguousarray(m[:P, :P].T) * f32(math.sqrt(HD))).astype(f32)

    def lhsT_tiles(wt, n_out_tiles, n_k_tiles):
        # wt: [K, Mout] -> [ot, p, kt, j] with [ot,p,kt,j] = wt[128*kt+p, 128*ot+j]
        a = wt.reshape(n_k_tiles, P, n_out_tiles, P)
        return np.ascontiguousarray(a.transpose(2, 1, 0, 3)).astype(NP_CDT)

    in_maps = []
    for r in range(CORES):
        ds = slice(DQ * r, DQ * (r + 1))
        wqT = wq[ds].T.copy()                      # [D, DQ]
        wkT = wk[ds].T.copy()
        for h in range(NHC):
            blk = slice(HD * h, HD * (h + 1))
            wqT[:, blk] = wqT[:, blk][:, perm]
            wkT[:, blk] = wkT[:, blk][:, perm]
        wqk = np.concatenate([lhsT_tiles(wqT, NHC, DT),
                              lhsT_tiles(wkT, NHC, DT)], axis=0)  # [8,P,DT,P]
        wvT = wv_e[ds].T.copy()                    # [D, DQ]
        w_v_l = np.ascontiguousarray(wvT.reshape(DT, P, DQ)).astype(NP_CDT)
        woT = wo[:, ds].T.copy()                   # [DQ, D]
        wo_l = lhsT_tiles(woT, 32, 4)              # [32, P, 4, P]
        wo_l = np.ascontiguousarray(wo_l.transpose(1, 0, 2, 3))  # [P,32,4,P]
        fs = slice(FC * r, FC * (r + 1))
        w1s = np.zeros((FP, D), f32)
        w3s = np.zeros((FP, D), f32)
        w1s[:FC] = w1[fs]
        w3s[:FC] = w3[fs]
        w1_l = lhsT_tiles(np.ascontiguousarray(w1s.T), FT, DT)  # [FT, P, DT, P]
        w3_l = lhsT_tiles(np.ascontiguousarray(w3s.T), FT, DT)
        w2s = np.zeros((FP, D), f32)
        w2s[:FC] = w2[:, fs].T                     # [FP, D] (rows = f)
        w2_l = lhsT_tiles(w2s, 32, FT)             # [32, P, FT, P]

        in_maps.append({
            "xT_s": np.ascontiguousarray(xT[ds]),
            "w_qk": wqk,
            "w_v": w_v_l,
            "w_o": wo_l,
            "w_1": w1_l,
            "w_3": w3_l,
            "w_2": w2_l,
            "cos2": cos2,
            "sinsg2": sinsg2,
            "dmask": dmask,
        })
    return in_maps


def kernel(x, freqs_cos, freqs_sin, mask, attn_norm_w, wq, wk, wv, wo,
           ffn_norm_w, w1, w2, w3, _trace=False):
    global _COMPILED
    if _COMPILED is None:
        _COMPILED = _build()
    nc = _COMPILED
    in_maps = _prep_inputs(x, freqs_cos, freqs_sin, mask, attn_norm_w,
                           wq, wk, wv, wo, ffn_norm_w, w1, w2, w3)
    res = run_bass_kernel_spmd(nc, in_maps, list(range(CORES)), trace=_trace)
    kernel.last_result = res
    outT = np.concatenate([res.results[r]["outT_s"] for r in range(CORES)],
                          axis=0)                  # [D, S]
    return np.ascontiguousarray(outT.T)[None].astype(np.float32)


# revision 13
# speedup vs baseline: 1.1575x; 1.0220x over previous
"""Llama-style transformer block on 8 TRN2 NeuronCores.

Megatron tensor-parallel with feature-major (transposed) activations:
  - Residual stream kept TRANSPOSED (x^T: [D, S]) so every matmul contracts
    over the partition dim with zero on-chip transposes.
  - Per core: 4 attention heads (512 of 4096 q/k/v dims) and 1376 (padded
    to 1408) of the 11008 FFN hidden dims.
  - RMSNorm: per-core partial sum-of-squares over the 512-feature shard,
    AllReduce [1,2048], scale own shard, AllGather normalized activations
    (feature-stacked = the exact layout the matmuls consume).
  - Attention: transposed scores ([s_k, s_q]) feed the AV matmul directly;
    softmax skips max-subtraction (exact; scores bounded).  Denominators
    via ones-matmul; normalization deferred to attention output.
  - RoPE via host-side even/odd permutation of head dims + two aligned
    half-tile swaps against [cos;cos] / [-sin;sin] tables.
  - wo / w2 partial sums -> token-chunked ReduceScatter, pipelined under
    the next chunk's compute.  AllGathers similarly chunked.

v2: everything pipelined in 512-token chunks to keep TensorE continuously
busy (HAM clock stays warm) and hide collectives under compute.
"""

import math

import ml_dtypes
import numpy as np

import concourse.bass as bass
import concourse.mybir as mybir
import concourse.tile as tile
from concourse import bacc
from concourse.bass_utils import run_bass_kernel_spmd

S = 2048
D = 4096
HD = 128
NH = 32
F = 11008
CORES = 8
NHC = NH // CORES          # heads per core = 4
DQ = NHC * HD              # q/k/v dims per core = 512
FC = F // CORES            # ffn dims per core = 1376
FT = 11                    # padded f-tiles per core
FP = FT * 128
EPS = 1e-5
P = 128
NCH = 4                    # 512-token chunks
CW = S // NCH              # chunk width = 512
DT = D // P                # d tiles = 32
ST = S // P                # s tiles = 16

CDT = mybir.dt.bfloat16
NP_CDT = ml_dtypes.bfloat16

_COMPILED = None


def _build():
    nc = bacc.Bacc("TRN2", target_bir_lowering=False, debug=False,
                   num_devices=CORES)
    f32 = mybir.dt.float32

    # ---- kernel I/O ----
    xT_s = nc.declare_dram_parameter("xT_s", [DQ, S], f32, isOutput=False)
    w_qk = nc.declare_dram_parameter("w_qk", [8, P, DT, P], CDT, isOutput=False)
    w_v = nc.declare_dram_parameter("w_v", [DT, P, DQ], CDT, isOutput=False)
    w_o = nc.declare_dram_parameter("w_o", [P, 32, 4, P], CDT, isOutput=False)
    w_1 = nc.declare_dram_parameter("w_1", [FT, P, DT, P], CDT, isOutput=False)
    w_3 = nc.declare_dram_parameter("w_3", [FT, P, DT, P], CDT, isOutput=False)
    w_2 = nc.declare_dram_parameter("w_2", [32, P, FT, P], CDT, isOutput=False)
    cos2 = nc.declare_dram_parameter("cos2", [P, S], CDT, isOutput=False)
    sinsg2 = nc.declare_dram_parameter("sinsg2", [P, S], CDT, isOutput=False)
    dmask = nc.declare_dram_parameter("dmask", [P, P], f32, isOutput=False)
    outT_s = nc.declare_dram_parameter("outT_s", [DQ, S], f32, isOutput=True)

    # ---- internal DRAM ----
    ssq1_in = nc.dram_tensor("ssq1_in", [1, S], f32)
    ssq1_out = nc.dram_tensor("ssq1_out", [1, S], f32, addr_space="Shared")
    zs_cc = [nc.dram_tensor(f"zs_cc{c}", [DQ, CW], CDT) for c in range(NCH)]
    zT_ag = [nc.dram_tensor(f"zT_ag{c}", [D, CW], CDT, addr_space="Shared")
             for c in range(NCH)]
    qt_dram = nc.dram_tensor("qt_dram", [DQ, S], CDT)
    kt_dram = nc.dram_tensor("kt_dram", [DQ, S], CDT)
    yT_cc = [nc.dram_tensor(f"yT_cc{c}", [D, CW], CDT) for c in range(NCH)]
    y_rs = [nc.dram_tensor(f"y_rs{c}", [DQ, CW], CDT) for c in range(NCH)]
    ssq2_in = nc.dram_tensor("ssq2_in", [1, S], f32)
    ssq2_out = nc.dram_tensor("ssq2_out", [1, S], f32, addr_space="Shared")
    hn_cc = [nc.dram_tensor(f"hn_cc{c}", [DQ, CW], CDT) for c in range(NCH)]
    hnT_ag = [nc.dram_tensor(f"hnT_ag{c}", [D, CW], CDT, addr_space="Shared")
              for c in range(NCH)]
    oT_cc = [nc.dram_tensor(f"oT_cc{c}", [D, CW], CDT) for c in range(NCH)]
    o_rs = [nc.dram_tensor(f"o_rs{c}", [DQ, CW], CDT) for c in range(NCH)]

    RG = [list(range(CORES))]
    ADD = mybir.AluOpType.add
    BYP = mybir.AluOpType.bypass
    EXP = mybir.ActivationFunctionType.Exp
    SQRT = mybir.ActivationFunctionType.Sqrt
    SILU = mybir.ActivationFunctionType.Silu
    ISQ = 1.0 / math.sqrt(HD)

    def ch(c):
        return slice(CW * c, CW * (c + 1))

    with tile.TileContext(nc) as tc:
        with (
            tc.tile_pool(name="persist", bufs=1) as persist,
            tc.tile_pool(name="ps_small", bufs=1, space="PSUM") as ps_small,
        ):
            ones = persist.tile([P, 1], CDT)
            nc.vector.memset(ones[:], 1.0)
            eps_sb = persist.tile([P, 1], f32)
            nc.vector.memset(eps_sb[:], EPS)
            dmask_sb = persist.tile([P, P], f32)
            nc.sync.dma_start(out=dmask_sb[:], in_=dmask[:])
            hT = [persist.tile([P, S], f32, tag=f"hT{i}", name=f"hT{i}")
                  for i in range(4)]

            # ============ stage 0: attn RMSNorm + AllGather(z) ============
            with tc.tile_pool(name="st0", bufs=1) as st0:
                xt = []
                for i in range(4):
                    t = st0.tile([P, S], f32, tag=f"xt{i}")
                    nc.sync.dma_start(out=t[:], in_=xT_s[P * i:P * (i + 1), :])
                    xt.append(t)
                sq = []
                for i in range(4):
                    t = st0.tile([P, S], CDT, tag=f"sq{i}")
                    nc.vector.tensor_mul(t[:], xt[i][:], xt[i][:])
                    sq.append(t)
                ssq_sb = st0.tile([1, S], f32)
                for c in range(NCH):
                    pt = ps_small.tile([1, CW], f32, tag="one512")
                    for i in range(4):
                        nc.tensor.matmul(pt[:], ones[:], sq[i][:, ch(c)],
                                         start=(i == 0), stop=(i == 3))
                    nc.any.tensor_copy(out=ssq_sb[:, ch(c)], in_=pt[:])
                nc.sync.dma_start(out=ssq1_in[:], in_=ssq_sb[:])
                nc.gpsimd.collective_compute(
                    "AllReduce", ADD, ins=[ssq1_in[:]], outs=[ssq1_out[:]],
                    replica_groups=RG)
                sg_sb = st0.tile([1, S], f32)
                nc.sync.dma_start(out=sg_sb[:], in_=ssq1_out[:])
                for c in range(NCH):
                    sr = st0.tile([1, CW], f32, tag="sr")
                    nc.scalar.activation(out=sr[:], in_=sg_sb[:, ch(c)],
                                         func=SQRT, bias=eps_sb[0:1],
                                         scale=1.0 / D)
                    nc.vector.reciprocal(out=sr[:], in_=sr[:])
                    srep = st0.tile([P, CW], f32, tag="srep")
                    nc.gpsimd.partition_broadcast(srep[:], sr[:])
                    for i in range(4):
                        z = st0.tile([P, CW], CDT, tag="zc")
                        nc.vector.tensor_mul(z[:], xt[i][:, ch(c)], srep[:])
                        nc.sync.dma_start(out=zs_cc[c][P * i:P * (i + 1), :],
                                          in_=z[:])
                    nc.gpsimd.collective_compute(
                        "AllGather", BYP, ins=[zs_cc[c][:]], outs=[zT_ag[c][:]],
                        replica_groups=RG)

            with tc.tile_pool(name="attn_persist", bufs=1) as apst:
                attnT = apst.tile([P, NHC, S], CDT)
                v_sb = apst.tile([P, ST, DQ], CDT)

                # ===== stage 1: Q/K/V projections (+RoPE), per 512-chunk ====
                with (
                    tc.tile_pool(name="st1", bufs=1) as st1,
                    tc.tile_pool(name="st1w", bufs=3) as st1w,
                    tc.tile_pool(name="st1z", bufs=2) as st1z,
                    tc.tile_pool(name="rope", bufs=3) as rope,
                    tc.tile_pool(name="ps_qkv", bufs=2, space="PSUM") as ps_qkv,
                    tc.tile_pool(name="ps_v", bufs=1, space="PSUM") as ps_v,
                ):
                    cos_sb = st1.tile([P, S], CDT, tag="cos")
                    sin_sb = st1.tile([P, S], CDT, tag="sin")
                    nc.sync.dma_start(out=cos_sb[:], in_=cos2[:])
                    nc.sync.dma_start(out=sin_sb[:], in_=sinsg2[:])
                    for c in range(NCH):
                      with nc.named_scope(f"qkv_c{c}"):
                        zt = st1z.tile([P, DT, CW], CDT, tag="zt")
                        zv = zT_ag[c][:].rearrange("(kt p) s -> p kt s", p=P)
                        nc.sync.dma_start(out=zt[:], in_=zv)
                        # --- Q and K ---
                        for ot in range(8):
                            wt = st1w.tile([P, DT, P], CDT, tag="wqk")
                            nc.sync.dma_start(out=wt[:], in_=w_qk[ot])
                            pt = ps_qkv.tile([P, CW], f32, tag="pqk")
                            for kt in range(DT):
                                nc.tensor.matmul(pt[:], wt[:, kt], zt[:, kt, :],
                                                 start=(kt == 0), stop=(kt == DT - 1))
                            # RoPE: out = pt*[c;c] + swap(pt)*[-s;s]
                            swp = rope.tile([P, CW], f32, tag="swp")
                            nc.vector.tensor_copy(swp[0:64, :], pt[64:128, :])
                            nc.vector.tensor_copy(swp[64:128, :], pt[0:64, :])
                            t1 = rope.tile([P, CW], f32, tag="t1")
                            t2 = rope.tile([P, CW], f32, tag="t2")
                            nc.vector.tensor_mul(t1[:], pt[:], cos_sb[:, ch(c)])
                            nc.vector.tensor_mul(t2[:], swp[:], sin_sb[:, ch(c)])
                            qk = rope.tile([P, CW], CDT, tag="qk")
                            nc.vector.tensor_add(qk[:], t1[:], t2[:])
                            dst = qt_dram if ot < 4 else kt_dram
                            hh = ot % 4
                            nc.sync.dma_start(out=dst[P * hh:P * (hh + 1), ch(c)],
                                              in_=qk[:])
                        # --- V: 4 token-tiles of this chunk ---
                        pts = [ps_v.tile([P, DQ], f32, tag=f"pv{i}", name=f"pv{i}")
                               for i in range(4)]
                        for kt in range(DT):
                            wv = st1w.tile([P, DQ], CDT, tag="wv")
                            nc.sync.dma_start(out=wv[:], in_=w_v[kt])
                            for i in range(4):
                                tok = slice(P * i, P * (i + 1))
                                nc.tensor.matmul(
                                    pts[i][:], zt[:, kt, tok], wv[:],
                                    start=(kt == 0), stop=(kt == DT - 1))
                        for i in range(4):
                            nc.any.tensor_copy(out=v_sb[:, 4 * c + i, :],
                                               in_=pts[i][:])

                # ====== stage 2+3: attention + wo + chunked RS(y) ======
                with (
                    tc.tile_pool(name="st2", bufs=4) as st2,
                    tc.tile_pool(name="st2qk", bufs=1) as st2qk,
                    tc.tile_pool(name="st2y", bufs=8) as st2y,
                    tc.tile_pool(name="exps", bufs=8) as exps,
                    tc.tile_pool(name="ps_sc", bufs=3, space="PSUM") as ps_sc,
                    tc.tile_pool(name="ps_av", bufs=2, space="PSUM") as ps_av,
                    tc.tile_pool(name="ps_wo", bufs=2, space="PSUM") as ps_wo,
                ):
                    wo_sb = st2qk.tile([P, 32, 4, P], CDT)
                    nc.sync.dma_start(out=wo_sb[:], in_=w_o[:])
                    qts, kts = [], []
                    for hh in range(NHC):
                        qt = st2qk.tile([P, S], CDT, tag=f"qt{hh}", name=f"qt{hh}")
                        kt_t = st2qk.tile([P, S], CDT, tag=f"kt{hh}", name=f"kt{hh}")
                        nc.sync.dma_start(out=qt[:],
                                          in_=qt_dram[P * hh:P * (hh + 1), :])
                        nc.sync.dma_start(out=kt_t[:],
                                          in_=kt_dram[P * hh:P * (hh + 1), :])
                        qts.append(qt)
                        kts.append(kt_t)
                    for qc in range(NCH):
                      with nc.named_scope(f"attn_c{qc}"):
                        nkt = 4 * qc + 4
                        for hh in range(NHC):
                            qt, kt_t = qts[hh], kts[hh]
                            avp = ps_av.tile([P, CW], f32, tag="avp")
                            smp = ps_small.tile([1, CW], f32, tag="one512")
                            for ktile in range(nkt):
                                diag = ktile >= 4 * qc
                                col0 = P * (ktile - 4 * qc) if diag else 0
                                scp = ps_sc.tile([P, CW], f32, tag="scp")
                                nc.tensor.matmul(
                                    scp[:, col0:],
                                    kt_t[:, P * ktile:P * (ktile + 1)],
                                    qt[:, CW * qc + col0:CW * (qc + 1)],
                                    start=True, stop=True)
                                if diag:
                                    nc.vector.tensor_add(
                                        scp[:, col0:col0 + P],
                                        scp[:, col0:col0 + P], dmask_sb[:])
                                et = exps.tile([P, CW], CDT, tag="et")
                                if col0 > 0:
                                    nc.vector.memset(et[:, 0:col0], 0.0)
                                nc.scalar.activation(out=et[:, col0:],
                                                     in_=scp[:, col0:],
                                                     func=EXP, scale=ISQ)
                                nc.tensor.matmul(
                                    avp[:], v_sb[:, ktile, P * hh:P * (hh + 1)],
                                    et[:], start=(ktile == 0),
                                    stop=(ktile == nkt - 1))
                                nc.tensor.matmul(smp[:], ones[:], et[:],
                                                 start=(ktile == 0),
                                                 stop=(ktile == nkt - 1))
                            rec = st2.tile([1, CW], f32, tag="rec")
                            nc.vector.reciprocal(out=rec[:], in_=smp[:])
                            rrep = st2.tile([P, CW], f32, tag="rrep")
                            nc.gpsimd.partition_broadcast(rrep[:], rec[:])
                            nc.vector.tensor_mul(attnT[:, hh, ch(qc)], avp[:],
                                                 rrep[:])
                        # ---- wo for this chunk, then RS it ----
                        for ot in range(32):
                            pt = ps_wo.tile([P, CW], f32, tag="pwo")
                            for dt_i in range(4):
                                nc.tensor.matmul(pt[:], wo_sb[:, ot, dt_i],
                                                 attnT[:, dt_i, ch(qc)],
                                                 start=(dt_i == 0), stop=(dt_i == 3))
                            yt = st2y.tile([P, CW], CDT, tag="yt")
                            if ot % 2 == 0:
                                nc.vector.tensor_copy(out=yt[:], in_=pt[:])
                            else:
                                nc.scalar.copy(out=yt[:], in_=pt[:])
                            nc.sync.dma_start(out=yT_cc[qc][P * ot:P * (ot + 1), :],
                                              in_=yt[:])
                        nc.gpsimd.collective_compute(
                            "ReduceScatter", ADD, ins=[yT_cc[qc][:]],
                            outs=[y_rs[qc][:]], replica_groups=RG)

            # ====== stage 4: residual + FFN RMSNorm + chunked AG(hn) ======
            with tc.tile_pool(name="st4", bufs=2) as st4:
                ssq_sb2 = persist.tile([1, S], f32)
                for c in range(NCH):
                    sq2 = []
                    for i in range(4):
                        xt_i = st4.tile([P, CW], f32, tag="x4")
                        nc.sync.dma_start(out=xt_i[:],
                                          in_=xT_s[P * i:P * (i + 1), ch(c)])
                        ys = st4.tile([P, CW], CDT, tag="ys")
                        nc.sync.dma_start(out=ys[:],
                                          in_=y_rs[c][P * i:P * (i + 1), :])
                        nc.vector.tensor_add(hT[i][:, ch(c)], xt_i[:], ys[:])
                        t = st4.tile([P, CW], CDT, tag="sq2")
                        nc.vector.tensor_mul(t[:], hT[i][:, ch(c)], hT[i][:, ch(c)])
                        sq2.append(t)
                    pt = ps_small.tile([1, CW], f32, tag="one512")
                    for i in range(4):
                        nc.tensor.matmul(pt[:], ones[:], sq2[i][:],
                                         start=(i == 0), stop=(i == 3))
                    nc.any.tensor_copy(out=ssq_sb2[:, ch(c)], in_=pt[:])
                nc.sync.dma_start(out=ssq2_in[:], in_=ssq_sb2[:])
                nc.gpsimd.collective_compute(
                    "AllReduce", ADD, ins=[ssq2_in[:]], outs=[ssq2_out[:]],
                    replica_groups=RG)
                sg2_sb = st4.tile([1, S], f32)
                nc.sync.dma_start(out=sg2_sb[:], in_=ssq2_out[:])
                for c in range(NCH):
                    sr2 = st4.tile([1, CW], f32, tag="sr2")
                    nc.scalar.activation(out=sr2[:], in_=sg2_sb[:, ch(c)],
                                         func=SQRT, bias=eps_sb[0:1],
                                         scale=1.0 / D)
                    nc.vector.reciprocal(out=sr2[:], in_=sr2[:])
                    srep2 = st4.tile([P, CW], f32, tag="srep2")
                    nc.gpsimd.partition_broadcast(srep2[:], sr2[:])
                    for i in range(4):
                        hn = st4.tile([P, CW], CDT, tag="hn4")
                        nc.vector.tensor_mul(hn[:], hT[i][:, ch(c)], srep2[:])
                        nc.sync.dma_start(out=hn_cc[c][P * i:P * (i + 1), :],
                                          in_=hn[:])
                    nc.gpsimd.collective_compute(
                        "AllGather", BYP, ins=[hn_cc[c][:]], outs=[hnT_ag[c][:]],
                        replica_groups=RG)

            # ============ stage 5: FFN + chunked RS(o) ============
            with (
                tc.tile_pool(name="st5w", bufs=2) as st5w,
                tc.tile_pool(name="st5w2", bufs=3) as st5w2,
                tc.tile_pool(name="st5h", bufs=2) as st5h,
                tc.tile_pool(name="st5g", bufs=2) as st5g,
                tc.tile_pool(name="st5t", bufs=4) as st5t,
                tc.tile_pool(name="ps_f1", bufs=2, space="PSUM") as ps_f1,
                tc.tile_pool(name="ps_f3", bufs=2, space="PSUM") as ps_f3,
                tc.tile_pool(name="ps_w2", bufs=2, space="PSUM") as ps_w2,
            ):
                for c in range(NCH):
                    with nc.named_scope(f"ffn_c{c}"):
                        hn_sb = st5h.tile([P, DT, CW], CDT, tag="hn")
                        hv = hnT_ag[c][:].rearrange("(kt p) s -> p kt s", p=P)
                        nc.sync.dma_start(out=hn_sb[:], in_=hv)
                        g_sb = st5g.tile([P, FT, CW], CDT, tag="g")
                        for ft in range(FT):
                            w1t = st5w.tile([P, DT, P], CDT, tag="w1")
                            w3t = st5w.tile([P, DT, P], CDT, tag="w3")
                            nc.sync.dma_start(out=w1t[:], in_=w_1[ft])
                            nc.sync.dma_start(out=w3t[:], in_=w_3[ft])
                            p1 = ps_f1.tile([P, CW], f32, tag="p1")
                            p3 = ps_f3.tile([P, CW], f32, tag="p3")
                            for kt in range(DT):
                                nc.tensor.matmul(p1[:], w1t[:, kt], hn_sb[:, kt, :],
                                                 start=(kt == 0), stop=(kt == DT - 1))
                            for kt in range(DT):
                                nc.tensor.matmul(p3[:], w3t[:, kt], hn_sb[:, kt, :],
                                                 start=(kt == 0), stop=(kt == DT - 1))
                            tsi = st5t.tile([P, CW], CDT, tag="tsi")
                            nc.scalar.activation(out=tsi[:], in_=p1[:], func=SILU)
                            nc.vector.tensor_mul(g_sb[:, ft, :], tsi[:], p3[:])
                        for ot in range(32):
                            w2t = st5w2.tile([P, FT, P], CDT, tag="w2")
                            nc.sync.dma_start(out=w2t[:], in_=w_2[ot])
                            pt = ps_w2.tile([P, CW], f32, tag="pw2")
                            for ft in range(FT):
                                nc.tensor.matmul(pt[:], w2t[:, ft], g_sb[:, ft, :],
                                                 start=(ft == 0), stop=(ft == FT - 1))
                            og = st5t.tile([P, CW], CDT, tag="og")
                            if ot % 2 == 0:
                                nc.vector.tensor_copy(out=og[:], in_=pt[:])
                            else:
                                nc.scalar.copy(out=og[:], in_=pt[:])
                            nc.sync.dma_start(out=oT_cc[c][P * ot:P * (ot + 1), :],
                                              in_=og[:])
                        nc.gpsimd.collective_compute(
                            "ReduceScatter", ADD, ins=[oT_cc[c][:]],
                            outs=[o_rs[c][:]], replica_groups=RG)

            # ============ stage 6: final residual ============
            with tc.tile_pool(name="st6", bufs=2) as st6:
                for c in range(NCH):
                    for i in range(4):
                        o_sb = st6.tile([P, CW], CDT, tag="osb")
                        nc.sync.dma_start(out=o_sb[:],
                                          in_=o_rs[c][P * i:P * (i + 1), :])
                        out_sb = st6.tile([P, CW], f32, tag="outsb")
                        nc.vector.tensor_add(out_sb[:], hT[i][:, ch(c)], o_sb[:])
                        nc.sync.dma_start(out=outT_s[P * i:P * (i + 1), ch(c)],
                                          in_=out_sb[:])

    nc.compile()
    return nc


def _prep_inputs(x, freqs_cos, freqs_sin, mask, attn_norm_w, wq, wk, wv, wo,
                 ffn_norm_w, w1, w2, w3):
    """Host-side sharding + weight layout. Returns in_maps for 8 cores."""
    f32 = np.float32
    x2 = np.asarray(x, f32)[0]                     # [S, D]
    xT = np.ascontiguousarray(x2.T)                # [D, S]
    anw = np.asarray(attn_norm_w, f32)
    fnw = np.asarray(ffn_norm_w, f32)
    wq = np.asarray(wq, f32) * anw[None, :]
    wk = np.asarray(wk, f32) * anw[None, :]
    wv_e = np.asarray(wv, f32)
    wo = np.asarray(wo, f32)
    w1 = np.asarray(w1, f32) * fnw[None, :]
    w3 = np.asarray(w3, f32) * fnw[None, :]
    w2 = np.asarray(w2, f32)

    perm = np.concatenate([np.arange(0, HD, 2), np.arange(1, HD, 2)])

    cosT = np.ascontiguousarray(np.asarray(freqs_cos, f32).T)   # [64, S]
    sinT = np.ascontiguousarray(np.asarray(freqs_sin, f32).T)
    cos2 = np.concatenate([cosT, cosT], axis=0).astype(NP_CDT)  # [128, S]
    sinsg2 = np.concatenate([-sinT, sinT], axis=0).astype(NP_CDT)
    m = np.asarray(mask, f32)[0, 0]
    dmask = (np.ascontiguousarray(m[:P, :P].T) * f32(math.sqrt(HD))).astype(f32)

    def lhsT_tiles(wt, n_out_tiles, n_k_tiles):
        # wt: [K, Mout] -> [ot, p, kt, j] with [ot,p,kt,j] = wt[128*kt+p, 128*ot+j]
        a = wt.reshape(n_k_tiles, P, n_out_tiles, P)
        return np.ascontiguousarray(a.transpose(2, 1, 0, 3)).astype(NP_CDT)

    in_maps = []
    for r in range(CORES):
        ds = slice(DQ * r, DQ * (r + 1))
        wqT = wq[ds].T.copy()                      # [D, DQ]
        wkT = wk[ds].T.copy()
        for h in range(NHC):
            blk = slice(HD * h, HD * (h + 1))
            wqT[:, blk] = wqT[:, blk][:, perm]
            wkT[:, blk] = wkT[:, blk][:, perm]
        wqk = np.concatenate([lhsT_tiles(wqT, NHC, DT),
                              lhsT_tiles(wkT, NHC, DT)], axis=0)  # [8,P,DT,P]
        wvT = wv_e[ds].T.copy()                    # [D, DQ]
        w_v_l = np.ascontiguousarray(wvT.reshape(DT, P, DQ)).astype(NP_CDT)
        woT = wo[:, ds].T.copy()                   # [DQ, D]
        wo_l = lhsT_tiles(woT, 32, 4)              # [32, P, 4, P]
        wo_l = np.ascontiguousarray(wo_l.transpose(1, 0, 2, 3))  # [P,32,4,P]
        fs = slice(FC * r, FC * (r + 1))
        w1s = np.zeros((FP, D), f32)
        w3s = np.zeros((FP, D), f32)
        w1s[:FC] = w1[fs]
        w3s[:FC] = w3[fs]
        w1_l = lhsT_tiles(np.ascontiguousarray(w1s.T), FT, DT)  # [FT, P, DT, P]
        w3_l = lhsT_tiles(np.ascontiguousarray(w3s.T), FT, DT)
        w2s = np.zeros((FP, D), f32)
        w2s[:FC] = w2[:, fs].T                     # [FP, D] (rows = f)
        w2_l = lhsT_tiles(w2s, 32, FT)             # [32, P, FT, P]

        in_maps.append({
            "xT_s": np.ascontiguousarray(xT[ds]),
            "w_qk": wqk,
            "w_v": w_v_l,
            "w_o": wo_l,
            "w_1": w1_l,
            "w_3": w3_l,
            "w_2": w2_l,
            "cos2": cos2,
            "sinsg2": sinsg2,
            "dmask": dmask,
        })
    return in_maps


def kernel(x, freqs_cos, freqs_sin, mask, attn_norm_w, wq, wk, wv, wo,
           ffn_norm_w, w1, w2, w3, _trace=False):
    global _COMPILED
    if _COMPILED is None:
        _COMPILED = _build()
    nc = _COMPILED
    in_maps = _prep_inputs(x, freqs_cos, freqs_sin, mask, attn_norm_w,
                           wq, wk, wv, wo, ffn_norm_w, w1, w2, w3)
    res = run_bass_kernel_spmd(nc, in_maps, list(range(CORES)), trace=_trace)
    kernel.last_result = res
    outT = np.concatenate([res.results[r]["outT_s"] for r in range(CORES)],
                          axis=0)                  # [D, S]
    return np.ascontiguousarray(outT.T)[None].astype(np.float32)
